# revision 1
# baseline (speedup 1.0000x reference)
"""Trainium2 Bass kernel for nn_GAT_MS (GAT+GCNII stack -> mean-shift attention stack).

Self-contained: takes full inputs, shards across 8 NeuronCores internally
(nodes row-sharded; edges partitioned by destination node), runs one SPMD
Bass/Tile program via run_bass_kernel_spmd, gathers the full output.
"""

import sys

try:
    import concourse.bass as _b  # noqa: F401
except ImportError:
    sys.path.insert(0, "/opt/trn_rl_repo")

import contextlib
import numpy as np
import ml_dtypes

import concourse.bass as bass  # noqa: F401
import concourse.bacc as bacc
import concourse.tile as tile
import concourse.mybir as mybir
from concourse.bass_utils import run_bass_kernel_spmd

F32 = mybir.dt.float32
BF16 = mybir.dt.bfloat16
I16 = mybir.dt.int16
U8 = mybir.dt.uint8
AF = mybir.ActivationFunctionType
ALU = mybir.AluOpType
AX = mybir.AxisListType

# ---- problem constants (hardcoded) ----
N = 4096
FEAT = 64
HID = 256
HEADS = 4
DH = 64
L_GNN = 4
L_MS = 4
LAMDA = 0.5
ALPHA = 0.1
NEG = -1e9
EPS = 1e-9

NC_ = 8            # cores
R = N // NC_       # rows per core = 512
NT = R // 128      # node tiles per core = 4
KC = HID // 128    # hid chunks = 2
CC = N // 512      # 512-wide column chunks = 8
JC = N // 128      # 128-wide j chunks = 32
SB = 16            # gather subblock (slots)
GCOLS = 384        # gathered row width (bf16): 256 H | 4 el_hi | 4 el_lo | pad
BIGC = 3.0e7       # distance-mask relu scale

_CACHE = {}

import os as _os
_NG = int(_os.environ.get("GATMS_NG", L_GNN))   # GAT layers to emit
_NM = int(_os.environ.get("GATMS_NM", L_MS))    # MS layers to emit
_P2 = int(_os.environ.get("GATMS_P2", 1))       # emit y/Bias0 phase
_GSUB = int(_os.environ.get("GATMS_GSUB", 0))   # 0=full GAT layer, 1=A+AG only, 2=+gather, 3=+agg noDVE


# ================= host-side preprocessing =================

def _prep_edges(src, dst):
    """Slot-major per-dst-tile edge layout. Returns (D, gidx[c], wmask[c])."""
    src = np.asarray(src).astype(np.int64)
    dst = np.asarray(dst).astype(np.int64)
    order = np.argsort(dst, kind="stable")
    s_s, d_s = src[order], dst[order]
    counts = np.bincount(d_s, minlength=N)
    deg_max = int(counts.max())
    D = max(SB, ((deg_max + SB - 1) // SB) * SB)
    starts = np.zeros(N + 1, np.int64)
    np.cumsum(counts, out=starts[1:])

    gidx_all, mask_all = [], []
    for c in range(NC_):
        # slot (t, s, j): position p = (t*D + s)*128 + j
        idx = np.full((NT, D, 128), R, np.int64)  # R=512 -> zero row of rank-0 block
        msk = np.zeros((NT, D, 128), bool)
        for t in range(NT):
            base = c * R + t * 128
            for j in range(128):
                a, b = starts[base + j], starts[base + j + 1]
                deg = b - a
                g = s_s[a:b]
                idx[t, :deg, j] = (g // R) * (R + 1) + (g % R)
                msk[t, :deg, j] = True
        flat = idx.reshape(-1)
        gi = np.zeros((128, len(flat) // 16), np.int16)
        pos = np.arange(len(flat))
        gi[pos % 16, pos // 16] = flat.astype(np.int16)
        for grp in range(1, 8):
            gi[16 * grp:16 * grp + 16] = gi[:16]
        wm = msk.transpose(2, 0, 1).reshape(128, NT * D).astype(ml_dtypes.bfloat16)
        gidx_all.append(np.ascontiguousarray(gi))
        mask_all.append(np.ascontiguousarray(wm))
    return D, gidx_all, mask_all


def _bf(x):
    return np.ascontiguousarray(np.asarray(x, np.float32).astype(ml_dtypes.bfloat16))


def _f32(x):
    return np.ascontiguousarray(np.asarray(x, np.float32))


def _prep_host(inputs):
    feat = _f32(inputs["feat"])
    xyz = _f32(inputs["xyz"])
    pair = (np.asarray(inputs["distance_mask"]) &
            np.asarray(inputs["big_inter_mask"])).astype(np.uint8)

    D, gidx, wmask = _prep_edges(inputs["src"], inputs["dst"])

    stat = {}
    stat["fcW"] = _f32(inputs["fc_W"])                      # [64, 256]
    stat["fcb"] = _f32(inputs["fc_b"]).reshape(HID)         # [256]
    stat["gatW"] = _bf(inputs["gat_W"])                     # [L, 256, 256]
    al = _f32(inputs["attn_l"]).reshape(L_GNN, 1, HID)
    ar = _f32(inputs["attn_r"]).reshape(L_GNN, 1, HID)
    stat["albc"] = _f32(np.broadcast_to(al, (L_GNN, 128, HID)))
    stat["arbc"] = _f32(np.broadcast_to(ar, (L_GNN, 128, HID)))
    stat["gcW"] = _bf(inputs["gcnii_W"])                    # [L, 512, 256]
    cgW = _f32(inputs["cls_gat_W"])
    cgb = _f32(inputs["cls_gat_b"])
    stat["wd"] = _f32(cgW[:, 1] - cgW[:, 0])                # [256]
    stat["bdbc"] = _f32(np.full((128, 1), float(cgb[1] - cgb[0])))
    for nm in ("q", "k", "v", "o"):
        stat[nm + "W"] = _bf(inputs[f"ms_{nm}_W"])          # [L, 256, 256]
    stat["qb"] = _f32(inputs["ms_q_b"]) / 16.0              # [L, 256]
    stat["kb"] = _f32(inputs["ms_k_b"])
    stat["ob"] = _f32(inputs["ms_o_b"])
    vb = _f32(inputs["ms_v_b"]).reshape(L_MS, 1, HID)
    stat["vbbc"] = _f32(np.broadcast_to(vb, (L_MS, 128, HID)))
    stat["clsW"] = _f32(inputs["cls_W"])                    # [256, 2]
    clsb = _f32(inputs["cls_b"]).reshape(1, 2)
    stat["clsbbc"] = _f32(np.broadcast_to(clsb, (128, 2)))
    stat["Ibf"] = _bf(np.eye(128))
    stat["If32"] = _f32(np.eye(128))

    in_maps = []
    for c in range(NC_):
        rows = slice(c * R, (c + 1) * R)
        m = dict(stat)
        m["featT"] = _f32(feat[rows].T)                     # [64, 512]
        m["xyz0"] = _f32(xyz[rows])                         # [512, 3]
        m["pmask"] = np.ascontiguousarray(pair[rows])       # [512, 4096] u8
        m["gidx"] = gidx[c]
        m["wmask"] = wmask[c]
        in_maps.append(m)
    return D, in_maps


# ================= device program =================

def _build_program(D):
    nc = bacc.Bacc("TRN2", target_bir_lowering=False, debug=False, num_devices=NC_)

    def din(name, shape, dt):
        return nc.dram_tensor(name, list(shape), dt, kind="ExternalInput").ap()

    T = {}
    T["featT_d"] = din("featT", (FEAT, R), F32)
    T["xyz0_d"] = din("xyz0", (R, 3), F32)
    T["pmask_d"] = din("pmask", (R, N), U8)
    T["gidx_d"] = din("gidx", (128, NT * D * 8), I16)
    T["wmask_d"] = din("wmask", (128, NT * D), BF16)
    T["fcW_d"] = din("fcW", (FEAT, HID), F32)
    T["fcb_d"] = din("fcb", (HID,), F32)
    T["gatW_d"] = din("gatW", (L_GNN, HID, HID), BF16)
    T["albc_d"] = din("albc", (L_GNN, 128, HID), F32)
    T["arbc_d"] = din("arbc", (L_GNN, 128, HID), F32)
    T["gcW_d"] = din("gcW", (L_GNN, 2 * HID, HID), BF16)
    T["wd_d"] = din("wd", (HID,), F32)
    T["bdbc_d"] = din("bdbc", (128, 1), F32)
    for nm in ("q", "k", "v", "o"):
        T[nm + "W_d"] = din(nm + "W", (L_MS, HID, HID), BF16)
    T["qb_d"] = din("qb", (L_MS, HID), F32)
    T["kb_d"] = din("kb", (L_MS, HID), F32)
    T["ob_d"] = din("ob", (L_MS, HID), F32)
    T["vbbc_d"] = din("vbbc", (L_MS, 128, HID), F32)
    T["clsW_d"] = din("clsW", (HID, 2), F32)
    T["clsbbc_d"] = din("clsbbc", (128, 2), F32)
    T["Ibf_d"] = din("Ibf", (128, 128), BF16)
    T["If32_d"] = din("If32", (128, 128), F32)

    T["out_d"] = nc.dram_tensor("out", [R, 2], F32, kind="ExternalOutput").ap()

    T["hext"] = [nc.dram_tensor(f"hext{l}", [R + 1, GCOLS], BF16).ap()
                 for l in range(L_GNN)]
    T["hfull"] = [nc.dram_tensor(f"hfull{l}", [NC_ * (R + 1), GCOLS], BF16,
                                 addr_space="Shared").ap() for l in range(L_GNN)]
    T["y_in"] = nc.dram_tensor("y_in", [R], F32).ap()
    T["y_out"] = nc.dram_tensor("y_out", [N], F32, addr_space="Shared").ap()
    T["bias0"] = nc.dram_tensor("bias0", [R, N], BF16).ap()
    KV = HID * R + R * HID
    T["KV"] = KV
    T["kv_in"] = [nc.dram_tensor(f"kv_in{l}", [KV], BF16).ap() for l in range(L_MS)]
    T["kv_out"] = [nc.dram_tensor(f"kv_out{l}", [NC_ * KV], BF16,
                                  addr_space="Shared").ap() for l in range(L_MS)]
    AUG = 3 * R + R + R * 3
    T["AUG"] = AUG
    T["aug_in"] = [nc.dram_tensor(f"aug_in{l}", [AUG], F32).ap() for l in range(L_MS)]
    T["aug_out"] = [nc.dram_tensor(f"aug_out{l}", [NC_ * AUG], F32,
                                   addr_space="Shared").ap() for l in range(L_MS)]

    THETA = [min(1.0, float(np.log(LAMDA / (l + 1) + 1.0))) for l in range(L_GNN)]

    with tile.TileContext(nc) as tc:
        _emit(nc, tc, D, THETA, T)
    nc.compile()
    return nc


def _emit(nc, tc, D, THETA, T):
    RG = [list(range(NC_))]
    KV = T["KV"]
    AUG = T["AUG"]
    with contextlib.ExitStack() as ctx:
        # ---------------- persistent SBUF ----------------
        pers = ctx.enter_context(tc.tile_pool(name="pers", bufs=1))

        def ptile(shape, dt, tag, src=None):
            t_ = pers.tile(list(shape), dt, tag=tag)
            if src is not None:
                nc.sync.dma_start(t_[:], src)
            return t_

        idx_s = ptile([128, NT * D * 8], I16, "idx", T["gidx_d"][:])
        wmask_s = ptile([128, NT * D], BF16, "wmask", T["wmask_d"][:])
        Ibf = ptile([128, 128], BF16, "Ibf", T["Ibf_d"][:])
        If32 = ptile([128, 128], F32, "If32", T["If32_d"][:])
        gatW_s = ptile([128, L_GNN, KC, HID], BF16, "gatW",
                       T["gatW_d"][:].rearrange("l (k p) h -> p l k h", p=128))
        albc_s = ptile([128, L_GNN, HID], F32, "albc",
                       T["albc_d"][:].rearrange("l p h -> p l h"))
        arbc_s = ptile([128, L_GNN, HID], F32, "arbc",
                       T["arbc_d"][:].rearrange("l p h -> p l h"))
        gcW_s = ptile([128, L_GNN, 4, HID], BF16, "gcW",
                      T["gcW_d"][:].rearrange("l (k p) h -> p l k h", p=128))
        msW_s = {nm: ptile([128, L_MS, KC, HID], BF16, nm + "W",
                           T[nm + "W_d"][:].rearrange("l (k p) h -> p l k h", p=128))
                 for nm in ("q", "k", "v", "o")}
        biases = {nm: ptile([128, L_MS, KC], F32, nm,
                            T[nm + "_d"][:].rearrange("l (k p) -> p l k", p=128))
                  for nm in ("qb", "kb", "ob")}
        vbbc_s = ptile([128, L_MS, HID], F32, "vbbc",
                       T["vbbc_d"][:].rearrange("l p h -> p l h"))
        fcb_s = ptile([128, KC], F32, "fcb",
                      T["fcb_d"][:].rearrange("(k p) -> p k", p=128))
        clsW_s = ptile([128, KC, 2], F32, "clsW",
                       T["clsW_d"][:].rearrange("(k p) c -> p k c", p=128))
        clsbbc_s = ptile([128, 2], F32, "clsbbc", T["clsbbc_d"][:])
        wd_s = ptile([128, KC], F32, "wd",
                     T["wd_d"][:].rearrange("(k p) -> p k", p=128))
        bdbc_s = ptile([128, 1], F32, "bdbc", T["bdbc_d"][:])
        featT_s = ptile([FEAT, R], F32, "featT", T["featT_d"][:])
        fcW_s = ptile([FEAT, HID], F32, "fcW", T["fcW_d"][:])

        # state
        xT = ptile([128, KC, R], F32, "xT")
        xTbf = ptile([128, KC, R], BF16, "xTbf")
        h0T = ptile([128, KC, R], F32, "h0T")
        h0Tbf = ptile([128, KC, R], BF16, "h0Tbf")
        er_all = ptile([128, NT, HEADS], F32, "er_all")
        xgT = ptile([128, KC, R], F32, "xgT")
        xgTbf = ptile([128, KC, R], BF16, "xgTbf")
        xyz_own = ptile([128, NT, 3], F32, "xyz_own")
        sq_own = ptile([128, NT], F32, "sq_own")
        XaugT = ptile([5, R], F32, "XaugT")
        YaugT = ptile([5, N], F32, "YaugT")
        kT_full = ptile([128, KC, N], BF16, "kT_full")
        VX = ptile([128, JC, 262], BF16, "VX")
        qT = ptile([128, KC, R], BF16, "qT")
        hmsT = ptile([128, KC, R], BF16, "hmsT")
        nbias = ptile([128, 1], F32, "nbias")   # -100*BIGC for the band relu
        nc.vector.memset(nbias[:], -100.0 * BIGC)
        zero384 = ptile([128, GCOLS], BF16, "zero384")
        nc.vector.memset(zero384[:], 0.0)
        # YaugT row 3 must stay 1.0; rows 0-2 and 4 are overwritten per layer.
        nc.vector.memset(YaugT[0:5, :], 1.0)

        # ---------------- P0: fc + relu ----------------
        with tc.tile_pool(name="p0ps", bufs=2, space="PSUM") as p0ps:
            for k in range(KC):
                ps = p0ps.tile([128, R], F32, tag="p0")
                nc.tensor.matmul(ps[:], fcW_s[:, k * 128:(k + 1) * 128], featT_s[:],
                                 start=True, stop=True)
                nc.scalar.activation(h0T[:, k, :], ps[:], AF.Relu,
                                     bias=fcb_s[:, k:k + 1], scale=1.0)
                nc.vector.tensor_copy(xT[:, k, :], h0T[:, k, :])
                nc.vector.tensor_copy(xTbf[:, k, :], h0T[:, k, :])
                nc.vector.tensor_copy(h0Tbf[:, k, :], h0T[:, k, :])

        # zero hext buffers (zero row 512 + pad cols + stale)
        for l in range(L_GNN):
            for rt in range(NT):
                nc.sync.dma_start(T["hext"][l][rt * 128:(rt + 1) * 128, :], zero384[:])
            nc.sync.dma_start(T["hext"][l][R:R + 1, :], zero384[0:1, :])

        # ---------------- P1: GAT + GCNII ----------------
        for l in range(_NG):
            hx, hf = T["hext"][l], T["hfull"][l]
            with tc.tile_pool(name=f"g{l}a", bufs=2, space="PSUM") as psA, \
                 tc.tile_pool(name=f"g{l}as", bufs=3) as sbA:
                for t in range(NT):
                    ph = psA.tile([128, HID], F32, tag="ph")
                    for k in range(KC):
                        nc.tensor.matmul(ph[:], xTbf[:, k, t * 128:(t + 1) * 128],
                                         gatW_s[:, l, k, :],
                                         start=(k == 0), stop=(k == KC - 1))
                    tmp = sbA.tile([128, HID], F32, tag="tmp")
                    el = sbA.tile([128, HEADS], F32, tag="el")
                    nc.vector.tensor_tensor(tmp[:], ph[:], albc_s[:, l, :], ALU.mult)
                    nc.vector.tensor_reduce(
                        el[:], tmp[:].rearrange("p (h d) -> p h d", h=HEADS),
                        axis=AX.X, op=ALU.add)
                    nc.vector.tensor_tensor(tmp[:], ph[:], arbc_s[:, l, :], ALU.mult)
                    nc.vector.tensor_reduce(
                        er_all[:, t, :], tmp[:].rearrange("p (h d) -> p h d", h=HEADS),
                        axis=AX.X, op=ALU.add)
                    hbf = sbA.tile([128, HID], BF16, tag="hbf")
                    nc.vector.tensor_copy(hbf[:], ph[:])
                    elhi = sbA.tile([128, HEADS], BF16, tag="elhi")
                    nc.vector.tensor_copy(elhi[:], el[:])
                    ello = sbA.tile([128, HEADS], BF16, tag="ello")
                    nc.vector.scalar_tensor_tensor(ello[:], elhi[:], -1.0, el[:],
                                                   op0=ALU.mult, op1=ALU.add)
                    rs = slice(t * 128, (t + 1) * 128)
                    nc.sync.dma_start(hx[rs, 0:HID], hbf[:])
                    nc.sync.dma_start(hx[rs, HID:HID + 4], elhi[:])
                    nc.sync.dma_start(hx[rs, HID + 4:HID + 8], ello[:])
            nc.gpsimd.collective_compute("AllGather", ALU.bypass, replica_groups=RG,
                                         ins=[hx.opt()], outs=[hf.opt()])
            if _GSUB == 1:
                continue
            with tc.tile_pool(name=f"g{l}b", bufs=2, space="PSUM") as psB, \
                 tc.tile_pool(name=f"g{l}bt", bufs=1, space="PSUM") as psT, \
                 tc.tile_pool(name=f"g{l}bs", bufs=2) as sbB, \
                 tc.tile_pool(name=f"g{l}bx", bufs=2) as sbX:
                rhsjunk = None
                if _GSUB == 2:
                    rhsjunk = sbB.tile([128, GCOLS], BF16, tag="rhsjunk")
                for t in range(NT):
                    if _GSUB == 2 and t > 0:
                        continue
                    agg = psB.tile([128, 260], F32, tag="agg")
                    for s0 in range(D // SB):
                        hg = sbB.tile([128, SB, GCOLS], BF16, tag="hg")
                        cb = (t * D + s0 * SB) * 8
                        nc.gpsimd.dma_gather(
                            out_ap=hg[:], in_ap=hf[:],
                            idxs_ap=idx_s[:, cb:cb + SB * 8],
                            num_idxs=SB * 128, num_idxs_reg=SB * 128,
                            elem_size=GCOLS, single_packet=False)
                        if _GSUB == 2:
                            nc.vector.tensor_copy(rhsjunk[:], hg[:, 0, :])
                            continue
                        e1 = sbB.tile([128, SB, HEADS], F32, tag="e1")
                        nc.vector.tensor_tensor(e1[:], hg[:, :, HID:HID + 4],
                                                hg[:, :, HID + 4:HID + 8], ALU.add)
                        erb = er_all[:, t, :].unsqueeze(1).to_broadcast((128, SB, HEADS))
                        nc.vector.tensor_tensor(e1[:], e1[:], erb, ALU.add)
                        ab = sbB.tile([128, SB, HEADS], F32, tag="ab")
                        nc.scalar.activation(ab[:], e1[:], AF.Abs, bias=0.0, scale=0.4)
                        nc.vector.scalar_tensor_tensor(e1[:], e1[:], 0.6, ab[:],
                                                       op0=ALU.mult, op1=ALU.add)
                        wbf = sbB.tile([128, SB, HEADS], BF16, tag="wbf")
                        nc.scalar.activation(wbf[:], e1[:], AF.Exp, bias=0.0, scale=1.0)
                        mk = wmask_s[:, t * D + s0 * SB:t * D + s0 * SB + SB]
                        nc.vector.tensor_tensor(
                            wbf[:], wbf[:],
                            mk.unsqueeze(-1).to_broadcast((128, SB, HEADS)), ALU.mult)
                        rhs = sbX.tile([128, SB, 260], BF16, tag="rhs")
                        nc.vector.tensor_tensor(
                            rhs[:, :, 0:HID].rearrange("p s (h d) -> p s h d", h=HEADS),
                            hg[:, :, 0:HID].rearrange("p s (h d) -> p s h d", h=HEADS),
                            wbf[:].unsqueeze(-1).to_broadcast((128, SB, HEADS, DH)),
                            ALU.mult)
                        nc.vector.tensor_copy(rhs[:, :, HID:HID + 4], wbf[:])
                        if _GSUB == 3:
                            continue
                        for s in range(SB):
                            nc.tensor.matmul(agg[:], Ibf[:], rhs[:, s, :],
                                             start=(s0 == 0 and s == 0),
                                             stop=(s0 == D // SB - 1 and s == SB - 1))
                    if _GSUB in (2, 3):
                        continue
                    zeps = sbB.tile([128, HEADS], F32, tag="zeps")
                    nc.vector.tensor_scalar_add(zeps[:], agg[:, HID:HID + 4], EPS)
                    inv4 = sbB.tile([128, HEADS], F32, tag="inv4")
                    nc.vector.reciprocal(inv4[:], zeps[:])
                    xgn = sbB.tile([128, HID], F32, tag="xgn")
                    nc.vector.tensor_tensor(
                        xgn[:].rearrange("p (h d) -> p h d", h=HEADS),
                        agg[:, 0:HID].rearrange("p (h d) -> p h d", h=HEADS),
                        inv4[:].unsqueeze(-1).to_broadcast((128, HEADS, DH)), ALU.mult)
                    xgnbf = sbB.tile([128, HID], BF16, tag="xgnbf")
                    nc.vector.tensor_copy(xgnbf[:], xgn[:])
                    for k in range(KC):
                        tpf = psT.tile([128, 128], F32, tag="tpf")
                        nc.tensor.transpose(tpf[:], xgn[:, k * 128:(k + 1) * 128], If32[:])
                        nc.vector.tensor_copy(xgT[:, k, t * 128:(t + 1) * 128], tpf[:])
                        tpb = psT.tile([128, 128], BF16, tag="tpb")
                        nc.tensor.transpose(tpb[:], xgnbf[:, k * 128:(k + 1) * 128], Ibf[:])
                        nc.vector.tensor_copy(xgTbf[:, k, t * 128:(t + 1) * 128], tpb[:])
            if _GSUB in (2, 3):
                continue
            th = THETA[l]
            with tc.tile_pool(name=f"g{l}c", bufs=2, space="PSUM") as psC, \
                 tc.tile_pool(name=f"g{l}cs", bufs=2) as sbC:
                for m in range(KC):
                    pg = psC.tile([128, R], F32, tag="pg")
                    for kc in range(4):
                        rhs_ = xgTbf[:, kc, :] if kc < KC else h0Tbf[:, kc - KC, :]
                        nc.tensor.matmul(pg[:], gcW_s[:, l, kc, m * 128:(m + 1) * 128],
                                         rhs_, start=(kc == 0), stop=(kc == 3))
                    u = sbC.tile([128, R], F32, tag="u")
                    nc.vector.scalar_tensor_tensor(u[:], pg[:], th, xT[:, m, :],
                                                   op0=ALU.mult, op1=ALU.add)
                    nc.vector.scalar_tensor_tensor(
                        u[:], xgT[:, m, :], (1.0 - th) * (1.0 - ALPHA), u[:],
                        op0=ALU.mult, op1=ALU.add)
                    nc.vector.scalar_tensor_tensor(
                        xT[:, m, :], h0T[:, m, :], (1.0 - th) * ALPHA, u[:],
                        op0=ALU.mult, op1=ALU.add)
                    nc.vector.tensor_copy(xTbf[:, m, :], xT[:, m, :])

        # ---------------- P2: y_hat + Bias0 ----------------
        if _P2:
            with tc.tile_pool(name="p2ps", bufs=2, space="PSUM") as p2ps, \
                 tc.tile_pool(name="p2tr", bufs=1, space="PSUM") as p2tr, \
                 tc.tile_pool(name="p2w", bufs=1) as p2w, \
                 tc.tile_pool(name="p2s", bufs=2) as p2s:
                yown = p2w.tile([128, NT], F32, tag="yown")
                for t in range(NT):
                    py = p2ps.tile([128, 1], F32, tag="py")
                    for k in range(KC):
                        nc.tensor.matmul(py[:], xT[:, k, t * 128:(t + 1) * 128],
                                         wd_s[:, k:k + 1], start=(k == 0), stop=(k == KC - 1))
                    nc.scalar.activation(yown[:, t:t + 1], py[:], AF.Sigmoid,
                                         bias=bdbc_s[:], scale=1.0)
                nc.sync.dma_start(T["y_in"][:].rearrange("(t p) -> p t", p=128), yown[:])
                nc.gpsimd.collective_compute("AllGather", ALU.bypass, replica_groups=RG,
                                             ins=[T["y_in"].opt()], outs=[T["y_out"].opt()])
                # d[i,j] = y_i - y_j via K=2: X2 = [y; -1], Y2 = [ones; y]
                Y2T = p2w.tile([2, N], F32, tag="Y2T")
                nc.vector.memset(Y2T[0:2, :], 1.0)
                nc.sync.dma_start(Y2T[1:2, :], T["y_out"][:].unsqueeze(0))
                X2T = p2w.tile([2, R], F32, tag="X2T")
                nc.vector.memset(X2T[0:2, :], -1.0)
                for t in range(NT):
                    tp1 = p2tr.tile([128, 128], F32, tag="tp1")
                    nc.tensor.transpose(tp1[0:1, 0:128], yown[:, t:t + 1], If32[:])
                    nc.vector.tensor_copy(X2T[0:1, t * 128:(t + 1) * 128], tp1[0:1, 0:128])
                for t in range(NT):
                    bsl = p2s.tile([128, N], BF16, tag="bsl")
                    pmt = p2s.tile([128, N], U8, tag="pmt")
                    nc.sync.dma_start(pmt[:], T["pmask_d"][t * 128:(t + 1) * 128, :])
                    for ci in range(CC):
                        cs = slice(ci * 512, (ci + 1) * 512)
                        pd = p2ps.tile([128, 512], F32, tag="pd")
                        nc.tensor.matmul(pd[:], X2T[:, t * 128:(t + 1) * 128], Y2T[:, cs],
                                         start=True, stop=True)
                        pmneg = p2s.tile([128, 512], F32, tag="pmneg")
                        nc.vector.tensor_scalar(pmneg[:], pmt[:, cs], 0.0, NEG,
                                                op0=ALU.is_equal, op1=ALU.mult)
                        ab2 = p2s.tile([128, 512], F32, tag="ab2")
                        nc.scalar.activation(ab2[:], pd[:], AF.Abs, bias=0.0, scale=1.0)
                        nc.vector.scalar_tensor_tensor(bsl[:, cs], ab2[:], -1.0,
                                                       pmneg[:], op0=ALU.mult, op1=ALU.add)
                    nc.sync.dma_start(T["bias0"][t * 128:(t + 1) * 128, :], bsl[:])

        # ---------------- P3: MS layers ----------------
        for l in range(_NM):
            with tc.tile_pool(name=f"m{l}x", bufs=1, space="PSUM") as psX, \
                 tc.tile_pool(name=f"m{l}xs", bufs=2) as sbXp:
                if l == 0:
                    nc.sync.dma_start(
                        xyz_own[:], T["xyz0_d"][:].rearrange("(t p) c -> p t c", p=128))
                    for t in range(NT):
                        sqv = sbXp.tile([128, 3], F32, tag="sqv")
                        nc.vector.tensor_tensor(sqv[:], xyz_own[:, t, :],
                                                xyz_own[:, t, :], ALU.mult)
                        nc.vector.tensor_reduce(sq_own[:, t:t + 1], sqv[:],
                                                axis=AX.X, op=ALU.add)
                asm = sbXp.tile([128, NT, 5], F32, tag="asm")
                nc.vector.tensor_copy(asm[:, :, 0:3], xyz_own[:])
                nc.vector.tensor_copy(asm[:, :, 3:4], sq_own[:].unsqueeze(-1))
                nc.vector.memset(asm[:, :, 4:5], 1.0)
                for t in range(NT):
                    tpx = psX.tile([128, 128], F32, tag="tpx")
                    nc.tensor.transpose(tpx[0:5, 0:128], asm[:, t, :], If32[:])
                    nc.vector.tensor_copy(XaugT[:, t * 128:(t + 1) * 128], tpx[0:5, 0:128])
                nc.sync.dma_start(
                    T["aug_in"][l][0:3 * R].rearrange("(c n) -> c n", c=3), XaugT[0:3, :])
                nc.sync.dma_start(
                    T["aug_in"][l][3 * R:4 * R].rearrange("(c n) -> c n", c=1),
                    XaugT[3:4, :])
                nc.sync.dma_start(
                    T["aug_in"][l][4 * R:].rearrange("(t p c) -> p t c", p=128, c=3),
                    xyz_own[:])
            with tc.tile_pool(name=f"m{l}q", bufs=2, space="PSUM") as psQ, \
                 tc.tile_pool(name=f"m{l}qs", bufs=2) as sbQ:
                for m in range(KC):
                    pq = psQ.tile([128, R], F32, tag="pq")
                    for k in range(KC):
                        nc.tensor.matmul(pq[:], msW_s["q"][:, l, k, m * 128:(m + 1) * 128],
                                         xTbf[:, k, :], start=(k == 0), stop=(k == KC - 1))
                    nc.scalar.activation(qT[:, m, :], pq[:], AF.Identity,
                                         bias=biases["qb"][:, l, m:m + 1], scale=1.0 / 16.0)
                    pk = psQ.tile([128, R], F32, tag="pq")
                    for k in range(KC):
                        nc.tensor.matmul(pk[:], msW_s["k"][:, l, k, m * 128:(m + 1) * 128],
                                         xTbf[:, k, :], start=(k == 0), stop=(k == KC - 1))
                    kbf = sbQ.tile([128, R], BF16, tag="kbf")
                    nc.scalar.activation(kbf[:], pk[:], AF.Identity,
                                         bias=biases["kb"][:, l, m:m + 1], scale=1.0)
                    nc.sync.dma_start(
                        T["kv_in"][l][m * 128 * R:(m + 1) * 128 * R]
                        .rearrange("(p n) -> p n", p=128), kbf[:])
                for t in range(NT):
                    pv = psQ.tile([128, HID], F32, tag="pv")
                    for k in range(KC):
                        nc.tensor.matmul(pv[:], xTbf[:, k, t * 128:(t + 1) * 128],
                                         msW_s["v"][:, l, k, :],
                                         start=(k == 0), stop=(k == KC - 1))
                    vbf = sbQ.tile([128, HID], BF16, tag="vbf")
                    nc.vector.tensor_tensor(vbf[:], pv[:], vbbc_s[:, l, :], ALU.add)
                    off = HID * R + t * 128 * HID
                    nc.sync.dma_start(
                        T["kv_in"][l][off:off + 128 * HID]
                        .rearrange("(p n) -> p n", p=128), vbf[:])
            nc.gpsimd.collective_compute("AllGather", ALU.bypass, replica_groups=RG,
                                         ins=[T["kv_in"][l].opt()],
                                         outs=[T["kv_out"][l].opt()])
            nc.gpsimd.collective_compute("AllGather", ALU.bypass, replica_groups=RG,
                                         ins=[T["aug_in"][l].opt()],
                                         outs=[T["aug_out"][l].opt()])
            with tc.tile_pool(name=f"m{l}u", bufs=2) as sbU:
                kvo, ago = T["kv_out"][l], T["aug_out"][l]
                for r in range(NC_):
                    for k in range(KC):
                        nc.sync.dma_start(
                            kT_full[:, k, r * R:(r + 1) * R],
                            kvo[r * KV + k * 128 * R:r * KV + (k + 1) * 128 * R]
                            .rearrange("(p n) -> p n", p=128))
                    nc.sync.dma_start(
                        VX[:, r * NT:(r + 1) * NT, 0:HID],
                        kvo[r * KV + HID * R:(r + 1) * KV]
                        .rearrange("(c p n) -> p c n", p=128, n=HID))
                    nc.sync.dma_start(
                        YaugT[0:3, r * R:(r + 1) * R],
                        ago[r * AUG:r * AUG + 3 * R].rearrange("(c n) -> c n", c=3))
                    nc.sync.dma_start(
                        YaugT[4:5, r * R:(r + 1) * R],
                        ago[r * AUG + 3 * R:r * AUG + 4 * R]
                        .rearrange("(c n) -> c n", c=1))
                nc.vector.tensor_scalar_mul(YaugT[0:3, :], YaugT[0:3, :], -2.0)
                xyzf = sbU.tile([128, JC, 3], F32, tag="xyzf")
                for r in range(NC_):
                    nc.sync.dma_start(
                        xyzf[:, r * NT:(r + 1) * NT, :],
                        ago[r * AUG + 4 * R:(r + 1) * AUG]
                        .rearrange("(t p c) -> p t c", p=128, c=3))
                xh = sbU.tile([128, JC, 3], BF16, tag="xh")
                nc.vector.tensor_copy(xh[:], xyzf[:])
                nc.vector.tensor_copy(VX[:, :, 256:259], xh[:])
                xl = sbU.tile([128, JC, 3], BF16, tag="xl")
                nc.vector.scalar_tensor_tensor(xl[:], xh[:], -1.0, xyzf[:],
                                               op0=ALU.mult, op1=ALU.add)
                nc.vector.tensor_copy(VX[:, :, 259:262], xl[:])
            with tc.tile_pool(name=f"m{l}r", bufs=2, space="PSUM") as psR, \
                 tc.tile_pool(name=f"m{l}rd", bufs=2, space="PSUM") as psD, \
                 tc.tile_pool(name=f"m{l}ro", bufs=2, space="PSUM") as psO, \
                 tc.tile_pool(name=f"m{l}rt", bufs=1, space="PSUM") as psTr, \
                 tc.tile_pool(name=f"m{l}rs", bufs=2) as sbR, \
                 tc.tile_pool(name=f"m{l}r1", bufs=1) as sbR1:
                for t in range(NT):
                    rsl = slice(t * 128, (t + 1) * 128)
                    bias_t = sbR.tile([128, N], BF16, tag="bias_t")
                    nc.sync.dma_start(bias_t[:], T["bias0"][rsl, :])
                    sc = sbR.tile([128, N], BF16, tag="sc")
                    for ci in range(CC):
                        cs = slice(ci * 512, (ci + 1) * 512)
                        ps_ = psR.tile([128, 512], F32, tag="ps_")
                        for k in range(KC):
                            nc.tensor.matmul(ps_[:], qT[:, k, rsl], kT_full[:, k, cs],
                                             start=(k == 0), stop=False)
                        nc.tensor.matmul(ps_[:], Ibf[:], bias_t[:, cs],
                                         start=False, stop=True)
                        pd2 = psD.tile([128, 512], F32, tag="pd2")
                        nc.tensor.matmul(pd2[:], XaugT[:, rsl], YaugT[:, cs],
                                         start=True, stop=True)
                        rlu = sbR.tile([128, 512], F32, tag="rlu")
                        nc.scalar.activation(rlu[:], pd2[:], AF.Relu,
                                             bias=nbias[:], scale=BIGC)
                        nc.vector.scalar_tensor_tensor(sc[:, cs], rlu[:], -1.0, ps_[:],
                                                       op0=ALU.mult, op1=ALU.add)
                    wexp = sbR1.tile([128, N], BF16, tag="wexp")
                    zrow = sbR1.tile([128, 1], F32, tag="zrow")
                    nc.scalar.activation(wexp[:], sc[:], AF.Exp, bias=0.0, scale=1.0,
                                         accum_out=zrow[:])
                    invz = sbR1.tile([128, 1], F32, tag="invz")
                    nc.vector.reciprocal(invz[:], zrow[:])
                    nc.vector.tensor_scalar_mul(wexp[:], wexp[:], invz[:])
                    attnT = sbR1.tile([128, JC, 128], BF16, tag="attnT")
                    nc.sync.dma_start_transpose(attnT[:], wexp[:])
                    po = psO.tile([128, 262], F32, tag="po")
                    for c in range(JC):
                        nc.tensor.matmul(po[:], attnT[:, c, :], VX[:, c, :],
                                         start=(c == 0), stop=(c == JC - 1))
                    hms = sbR1.tile([128, HID], BF16, tag="hms")
                    nc.vector.tensor_copy(hms[:], po[:, 0:HID])
                    for k in range(KC):
                        tph = psTr.tile([128, 128], BF16, tag="tph")
                        nc.tensor.transpose(tph[:], hms[:, k * 128:(k + 1) * 128], Ibf[:])
                        nc.vector.tensor_copy(hmsT[:, k, rsl], tph[:])
                    x6 = sbR1.tile([128, 6], F32, tag="x6")
                    nc.vector.tensor_copy(x6[:], po[:, 256:262])
                    nc.vector.tensor_tensor(xyz_own[:, t, :], x6[:, 0:3], x6[:, 3:6],
                                            ALU.add)
                    sqv2 = sbR1.tile([128, 3], F32, tag="sqv2")
                    nc.vector.tensor_tensor(sqv2[:], xyz_own[:, t, :],
                                            xyz_own[:, t, :], ALU.mult)
                    nc.vector.tensor_reduce(sq_own[:, t:t + 1], sqv2[:],
                                            axis=AX.X, op=ALU.add)
            with tc.tile_pool(name=f"m{l}o", bufs=2, space="PSUM") as psP:
                for m in range(KC):
                    pp = psP.tile([128, R], F32, tag="pp")
                    for k in range(KC):
                        nc.tensor.matmul(pp[:], msW_s["o"][:, l, k, m * 128:(m + 1) * 128],
                                         hmsT[:, k, :], start=(k == 0), stop=(k == KC - 1))
                    nc.vector.scalar_tensor_tensor(
                        xT[:, m, :], pp[:], biases["ob"][:, l, m:m + 1], xT[:, m, :],
                        op0=ALU.add, op1=ALU.add)
                    nc.vector.tensor_copy(xTbf[:, m, :], xT[:, m, :])

        # ---------------- P4: final logits ----------------
        with tc.tile_pool(name="p4ps", bufs=2, space="PSUM") as p4ps, \
             tc.tile_pool(name="p4s", bufs=2) as p4s:
            for t in range(NT):
                pf = p4ps.tile([128, 2], F32, tag="pf")
                for k in range(KC):
                    nc.tensor.matmul(pf[:], xT[:, k, t * 128:(t + 1) * 128],
                                     clsW_s[:, k, :], start=(k == 0), stop=(k == KC - 1))
                ot = p4s.tile([128, 2], F32, tag="ot")
                nc.vector.tensor_tensor(ot[:], pf[:], clsbbc_s[:], ALU.add)
                nc.sync.dma_start(T["out_d"][t * 128:(t + 1) * 128, :], ot[:])


# ================= entry point =================

def kernel(**inputs) -> np.ndarray:
    D, in_maps = _prep_host(inputs)
    if D not in _CACHE:
        _CACHE[D] = _build_program(D)
    nc = _CACHE[D]
    res = run_bass_kernel_spmd(nc, in_maps, list(range(NC_)))
    out = np.concatenate([res.results[c]["out"] for c in range(NC_)], axis=0)
    return np.ascontiguousarray(out.astype(np.float32))



# revision 8
# speedup vs baseline: 2.0009x; 2.0009x over previous
"""Trainium2 Bass kernel for nn_GAT_MS (GAT+GCNII stack -> mean-shift attention stack).

Dense formulation: GAT edge-softmax is computed as a dense masked [N x R]
transposed score matrix per head (rank-4 bf16 matmuls from hi/lo-split
attention logits), with edge multiplicity folded in as a count-matrix
multiply after exp. Aggregation is a dense matmul with a ones-column to
carry the softmax denominator. The mean-shift layers use the same
transposed-score layout (scores[j,i]) so no attention transpose is needed;
the distance-band test rides a K=13 bf16 hi/lo rank matmul and the
delta-y/pair-mask bias is pre-exponentiated (expB) so it folds in as a
single gpsimd multiply.

Self-contained: takes full inputs, shards nodes across 8 NeuronCores,
runs one SPMD Bass/Tile program via run_bass_kernel_spmd, gathers output.
"""

import sys

try:
    import concourse.bass as _b  # noqa: F401
except ImportError:
    sys.path.insert(0, "/opt/trn_rl_repo")

import contextlib
import os as _os
import numpy as np
import ml_dtypes

import concourse.bass as bass  # noqa: F401
import concourse.bacc as bacc
import concourse.tile as tile
import concourse.mybir as mybir
from concourse.bass_utils import run_bass_kernel_spmd

F32 = mybir.dt.float32
BF16 = mybir.dt.bfloat16
U8 = mybir.dt.uint8
AF = mybir.ActivationFunctionType
ALU = mybir.AluOpType
AX = mybir.AxisListType

# ---- problem constants (hardcoded) ----
N = 4096
FEAT = 64
HID = 256
HEADS = 4
DH = 64
L_GNN = 4
L_MS = 4
LAMDA = 0.5
ALPHA = 0.1
NEG = -1e9
EPS = 1e-9
BIGC = 3.0e7          # distance-band relu scale
BAND2 = 100.0         # band_width^2

NC_ = 8               # cores
R = N // NC_          # rows per core = 512
NT = R // 128         # node tiles per core = 4
JT = N // 128         # j tiles = 32
KC = HID // 128       # hid chunks = 2

# GAT hext layout: per node row [H0|1|H1|1|H2|1|H3|1|el8] = 268 cols
HEXTW = HEADS * (DH + 1) + 8          # 268
OFF_ELT = R * HEXTW                   # elT8 [8, R] row-major
FLAT_G = R * HEXTW + 8 * R

# MS kv flat layout (bf16)
VW = HID + 7                          # v(256)|xyz_hi(3)|xyz_lo(3)|1 = 263
OFF_K = 0
OFF_V = KC * 128 * R                  # 131072
OFF_XJ = OFF_V + NT * 128 * VW        # 265728
FLAT_M = OFF_XJ + 13 * R              # 272384

_NG = int(_os.environ.get("GATMS_NG", L_GNN))
_NM = int(_os.environ.get("GATMS_NM", L_MS))

_CACHE = {}


# ================= host-side preprocessing =================

def _bf(x):
    return np.ascontiguousarray(np.asarray(x, np.float32).astype(ml_dtypes.bfloat16))


def _f32(x):
    return np.ascontiguousarray(np.asarray(x, np.float32))


def _prep_host(inputs):
    feat = _f32(inputs["feat"])
    xyz = _f32(inputs["xyz"])
    pair = (np.asarray(inputs["distance_mask"]) &
            np.asarray(inputs["big_inter_mask"])).astype(np.uint8)
    src = np.asarray(inputs["src"]).astype(np.int64)
    dst = np.asarray(inputs["dst"]).astype(np.int64)
    cnt = np.zeros((N, N), np.int32)
    np.add.at(cnt, (dst, src), 1)
    cntT = _bf(cnt.T)                                    # [src j, dst i]

    stat = {}
    stat["fcW"] = _f32(inputs["fc_W"])
    stat["fcb"] = _f32(inputs["fc_b"]).reshape(HID)
    stat["gatW"] = _bf(inputs["gat_W"])
    al = _f32(inputs["attn_l"]).reshape(L_GNN, 1, HID)
    ar = _f32(inputs["attn_r"]).reshape(L_GNN, 1, HID)
    stat["albc"] = _f32(np.broadcast_to(al, (L_GNN, 128, HID)))
    stat["arbc"] = _f32(np.broadcast_to(ar, (L_GNN, 128, HID)))
    stat["gcW"] = _bf(inputs["gcnii_W"])
    cgW = _f32(inputs["cls_gat_W"])
    cgb = _f32(inputs["cls_gat_b"])
    stat["wd"] = _f32(cgW[:, 1] - cgW[:, 0])
    stat["bdbc"] = _f32(np.full((128, 1), float(cgb[1] - cgb[0])))
    for nm in ("q", "k", "v", "o"):
        stat[nm + "W"] = _bf(inputs[f"ms_{nm}_W"])
    stat["qb"] = _f32(inputs["ms_q_b"]) / 16.0
    stat["kb"] = _f32(inputs["ms_k_b"])
    stat["ob"] = _f32(inputs["ms_o_b"])
    vb = _f32(inputs["ms_v_b"]).reshape(L_MS, 1, HID)
    stat["vbbc"] = _f32(np.broadcast_to(vb, (L_MS, 128, HID)))
    stat["clsW"] = _f32(inputs["cls_W"])
    clsb = _f32(inputs["cls_b"]).reshape(1, 2)
    stat["clsbbc"] = _f32(np.broadcast_to(clsb, (128, 2)))
    stat["Ibf"] = _bf(np.eye(128))
    stat["If32"] = _f32(np.eye(128))

    in_maps = []
    for c in range(NC_):
        rows = slice(c * R, (c + 1) * R)
        m = dict(stat)
        m["featT"] = _f32(feat[rows].T)                  # [64, 512]
        m["xyz0"] = _f32(xyz[rows])                      # [512, 3]
        m["pairT"] = np.ascontiguousarray(pair[rows].T)  # [4096, 512] u8
        m["cntT"] = np.ascontiguousarray(cntT[:, rows])  # [4096, 512] bf16
        in_maps.append(m)
    return in_maps


# ================= device program =================

def _build_program():
    nc = bacc.Bacc("TRN2", target_bir_lowering=False, debug=False, num_devices=NC_)

    def din(name, shape, dt):
        return nc.dram_tensor(name, list(shape), dt, kind="ExternalInput").ap()

    T = {}
    T["featT_d"] = din("featT", (FEAT, R), F32)
    T["xyz0_d"] = din("xyz0", (R, 3), F32)
    T["pairT_d"] = din("pairT", (N, R), U8)
    T["cntT_d"] = din("cntT", (N, R), BF16)
    T["fcW_d"] = din("fcW", (FEAT, HID), F32)
    T["fcb_d"] = din("fcb", (HID,), F32)
    T["gatW_d"] = din("gatW", (L_GNN, HID, HID), BF16)
    T["albc_d"] = din("albc", (L_GNN, 128, HID), F32)
    T["arbc_d"] = din("arbc", (L_GNN, 128, HID), F32)
    T["gcW_d"] = din("gcW", (L_GNN, 2 * HID, HID), BF16)
    T["wd_d"] = din("wd", (HID,), F32)
    T["bdbc_d"] = din("bdbc", (128, 1), F32)
    for nm in ("q", "k", "v", "o"):
        T[nm + "W_d"] = din(nm + "W", (L_MS, HID, HID), BF16)
    T["qb_d"] = din("qb", (L_MS, HID), F32)
    T["kb_d"] = din("kb", (L_MS, HID), F32)
    T["ob_d"] = din("ob", (L_MS, HID), F32)
    T["vbbc_d"] = din("vbbc", (L_MS, 128, HID), F32)
    T["clsW_d"] = din("clsW", (HID, 2), F32)
    T["clsbbc_d"] = din("clsbbc", (128, 2), F32)
    T["Ibf_d"] = din("Ibf", (128, 128), BF16)
    T["If32_d"] = din("If32", (128, 128), F32)

    T["out_d"] = nc.dram_tensor("out", [R, 2], F32, kind="ExternalOutput").ap()

    T["hext"] = [nc.dram_tensor(f"hext{l}", [FLAT_G], BF16).ap()
                 for l in range(L_GNN)]
    T["hfull"] = [nc.dram_tensor(f"hfull{l}", [NC_ * FLAT_G], BF16,
                                 addr_space="Shared").ap() for l in range(L_GNN)]
    T["yt2_in"] = nc.dram_tensor("yt2_in", [2 * R], BF16).ap()
    T["yt2_out"] = nc.dram_tensor("yt2_out", [NC_ * 2 * R], BF16,
                                  addr_space="Shared").ap()
    T["expB_d"] = nc.dram_tensor("expB", [N, R], BF16).ap()
    T["kv_in"] = [nc.dram_tensor(f"kv_in{l}", [FLAT_M], BF16).ap()
                  for l in range(L_MS)]
    T["kv_out"] = [nc.dram_tensor(f"kv_out{l}", [NC_ * FLAT_M], BF16,
                                  addr_space="Shared").ap() for l in range(L_MS)]

    THETA = [min(1.0, float(np.log(LAMDA / (l + 1) + 1.0))) for l in range(L_GNN)]

    with tile.TileContext(nc) as tc:
        _emit(nc, tc, THETA, T)
    nc.compile()
    return nc


def _emit(nc, tc, THETA, T):
    RG = [list(range(NC_))]
    with contextlib.ExitStack() as ctx:
        pers = ctx.enter_context(tc.tile_pool(name="pers", bufs=1))

        def ptile(shape, dt, tag, src=None):
            t_ = pers.tile(list(shape), dt, tag=tag)
            if src is not None:
                nc.sync.dma_start(t_[:], src)
            return t_

        Ibf = ptile([128, 128], BF16, "Ibf", T["Ibf_d"][:])
        If32 = ptile([128, 128], F32, "If32", T["If32_d"][:])
        gatW_s = ptile([128, L_GNN, KC, HID], BF16, "gatW",
                       T["gatW_d"][:].rearrange("l (k p) h -> p l k h", p=128))
        albc_s = ptile([128, L_GNN, HID], F32, "albc",
                       T["albc_d"][:].rearrange("l p h -> p l h"))
        arbc_s = ptile([128, L_GNN, HID], F32, "arbc",
                       T["arbc_d"][:].rearrange("l p h -> p l h"))
        gcW_s = ptile([128, L_GNN, 4, HID], BF16, "gcW",
                      T["gcW_d"][:].rearrange("l (k p) h -> p l k h", p=128))
        msW_s = {}
        for nm in ("q", "k", "v", "o"):
            w_ = pers.tile([128, L_MS, KC, HID], BF16, tag=nm + "W", name=nm + "W_s")
            nc.sync.dma_start(w_[:], T[nm + "W_d"][:]
                              .rearrange("l (k p) h -> p l k h", p=128))
            msW_s[nm] = w_
        biases = {}
        for nm in ("qb", "kb", "ob"):
            b_ = pers.tile([128, L_MS, KC], F32, tag=nm, name=nm + "_s")
            nc.sync.dma_start(b_[:], T[nm + "_d"][:]
                              .rearrange("l (k p) -> p l k", p=128))
            biases[nm] = b_
        vbbc_s = ptile([128, L_MS, HID], F32, "vbbc",
                       T["vbbc_d"][:].rearrange("l p h -> p l h"))
        fcb_s = ptile([128, KC], F32, "fcb",
                      T["fcb_d"][:].rearrange("(k p) -> p k", p=128))
        clsW_s = ptile([128, KC, 2], F32, "clsW",
                       T["clsW_d"][:].rearrange("(k p) c -> p k c", p=128))
        clsbbc_s = ptile([128, 2], F32, "clsbbc", T["clsbbc_d"][:])
        wd_s = ptile([128, KC], F32, "wd",
                     T["wd_d"][:].rearrange("(k p) -> p k", p=128))
        bdbc_s = ptile([128, 1], F32, "bdbc", T["bdbc_d"][:])
        # state (persistent)
        xT = ptile([128, KC, R], F32, "xT")
        xTbf = ptile([128, KC, R], BF16, "xTbf")
        h0T = ptile([128, KC, R], F32, "h0T")
        h0Tbf = ptile([128, KC, R], BF16, "h0Tbf")
        xgT = ptile([128, KC, R], F32, "xgT")
        xgTbf = ptile([128, KC, R], BF16, "xgTbf")
        ones1f = ptile([1, 64], F32, "ones1f")
        nc.vector.memset(ones1f[:], 1.0)
        nbias = ptile([128, 1], F32, "nbias")
        nc.vector.memset(nbias[:], -BAND2 * BIGC)

        p0stk = contextlib.ExitStack()
        p0pool = p0stk.enter_context(tc.tile_pool(name="p0pool", bufs=1))
        featT_s = p0pool.tile([FEAT, R], F32, tag="featT", name="featT_s")
        nc.sync.dma_start(featT_s[:], T["featT_d"][:])
        fcW_s = p0pool.tile([FEAT, HID], F32, tag="fcW", name="fcW_s")
        nc.sync.dma_start(fcW_s[:], T["fcW_d"][:])

        # ---------------- P0: fc + relu ----------------
        with tc.tile_pool(name="p0ps", bufs=2, space="PSUM") as p0ps:
            for k in range(KC):
                ps = p0ps.tile([128, R], F32, tag="p0")
                nc.tensor.matmul(ps[:], fcW_s[:, k * 128:(k + 1) * 128], featT_s[:],
                                 start=True, stop=True)
                nc.scalar.activation(h0T[:, k, :], ps[:], AF.Relu,
                                     bias=fcb_s[:, k:k + 1], scale=1.0)
                nc.vector.tensor_copy(xT[:, k, :], h0T[:, k, :])
                nc.vector.tensor_copy(xTbf[:, k, :], h0T[:, k, :])
                nc.vector.tensor_copy(h0Tbf[:, k, :], h0T[:, k, :])

        p0stk.close()

        # ---------------- P1: GAT + GCNII ----------------
        gatstk = contextlib.ExitStack()
        gatp = gatstk.enter_context(tc.tile_pool(name="gatp", bufs=1))
        C_s = gatp.tile([128, JT, R], BF16, tag="C_s", name="C_s")
        nc.sync.dma_start(C_s[:], T["cntT_d"][:].rearrange("(t p) n -> p t n", p=128))
        Haug_s = gatp.tile([128, JT, HEXTW], BF16, tag="Haug", name="Haug_s")
        elT = [gatp.tile([4, N], BF16, tag=f"elT{h}", name=f"elT{h}")
               for h in range(HEADS)]
        erT = [gatp.tile([4, R], BF16, tag=f"erT{h}", name=f"erT{h}")
               for h in range(HEADS)]
        for h in range(HEADS):
            nc.vector.memset(elT[h][:], 1.0)
            nc.vector.memset(erT[h][0:2, :], 1.0)
        for l in range(_NG):
            hx, hf = T["hext"][l], T["hfull"][l]
            with tc.tile_pool(name=f"g{l}a", bufs=2, space="PSUM") as psA, \
                 tc.tile_pool(name=f"g{l}at", bufs=2, space="PSUM") as psAT, \
                 tc.tile_pool(name=f"g{l}as", bufs=2) as sbA, \
                 tc.tile_pool(name=f"g{l}ae", bufs=1) as sbE:
                el8 = sbE.tile([128, NT, 8], BF16, tag="el8")
                er8 = sbE.tile([128, NT, 8], BF16, tag="er8")
                elT8 = sbE.tile([8, R], BF16, tag="elT8")
                erT8 = sbE.tile([8, R], BF16, tag="erT8")
                for t in range(NT):
                    tsl = slice(t * 128, (t + 1) * 128)
                    ph = psA.tile([128, HID], F32, tag="ph")
                    for k in range(KC):
                        nc.tensor.matmul(ph[:], xTbf[:, k, tsl],
                                         gatW_s[:, l, k, :],
                                         start=(k == 0), stop=(k == KC - 1))
                    tmp = sbA.tile([128, HID], F32, tag="tmp")
                    e4 = sbA.tile([128, 2, HEADS], F32, tag="e4")
                    nc.vector.tensor_tensor(tmp[:], ph[:], albc_s[:, l, :], ALU.mult)
                    nc.vector.tensor_reduce(
                        e4[:, 0, :], tmp[:].rearrange("p (h d) -> p h d", h=HEADS),
                        axis=AX.X, op=ALU.add)
                    nc.vector.tensor_tensor(tmp[:], ph[:], arbc_s[:, l, :], ALU.mult)
                    nc.vector.tensor_reduce(
                        e4[:, 1, :], tmp[:].rearrange("p (h d) -> p h d", h=HEADS),
                        axis=AX.X, op=ALU.add)
                    ehi = sbA.tile([128, 2, HEADS], BF16, tag="ehi")
                    nc.vector.tensor_copy(ehi[:], e4[:])
                    elo = sbA.tile([128, 2, HEADS], BF16, tag="elo")
                    nc.vector.scalar_tensor_tensor(elo[:], ehi[:], -1.0, e4[:],
                                                   op0=ALU.mult, op1=ALU.add)
                    e8v = el8[:, t, :].rearrange("p (h k) -> p h k", k=2)
                    nc.vector.tensor_copy(e8v[:, :, 0:1], ehi[:, 0, :].unsqueeze(-1))
                    nc.vector.tensor_copy(e8v[:, :, 1:2], elo[:, 0, :].unsqueeze(-1))
                    r8v = er8[:, t, :].rearrange("p (h k) -> p h k", k=2)
                    nc.vector.tensor_copy(r8v[:, :, 0:1], ehi[:, 1, :].unsqueeze(-1))
                    nc.vector.tensor_copy(r8v[:, :, 1:2], elo[:, 1, :].unsqueeze(-1))
                    stage = sbA.tile([128, HEXTW], BF16, tag="stage")
                    for h in range(HEADS):
                        nc.vector.tensor_copy(
                            stage[:, h * 65:h * 65 + 64], ph[:, h * 64:(h + 1) * 64])
                        nc.vector.memset(stage[:, h * 65 + 64:h * 65 + 65], 1.0)
                    nc.vector.tensor_copy(stage[:, 260:268], el8[:, t, :])
                    nc.sync.dma_start(
                        hx[t * 128 * HEXTW:(t + 1) * 128 * HEXTW]
                        .rearrange("(p c) -> p c", p=128), stage[:])
                for t in range(NT):
                    tsl = slice(t * 128, (t + 1) * 128)
                    pt1 = psAT.tile([8, 128], BF16, tag="pt1")
                    nc.tensor.transpose(pt1[:], el8[:, t, :], Ibf[:])
                    nc.vector.tensor_copy(elT8[:, tsl], pt1[:])
                    pt2 = psAT.tile([8, 128], BF16, tag="pt2")
                    nc.tensor.transpose(pt2[:], er8[:, t, :], Ibf[:])
                    nc.vector.tensor_copy(erT8[:, tsl], pt2[:])
                nc.sync.dma_start(
                    hx[OFF_ELT:OFF_ELT + 8 * R].rearrange("(k n) -> k n", k=8),
                    elT8[:])
                for h in range(HEADS):
                    nc.sync.dma_start(erT[h][2:4, :], erT8[2 * h:2 * h + 2, :])
            nc.gpsimd.collective_compute("AllGather", ALU.bypass, replica_groups=RG,
                                         ins=[hx.opt()], outs=[hf.opt()])
            with tc.tile_pool(name=f"g{l}u", bufs=1) as sbU:  # noqa: F841
                hfr = hf.rearrange("(r x) -> r x", r=NC_)
                for r in range(NC_):
                    nc.sync.dma_start(
                        Haug_s[:, NT * r:NT * (r + 1), :],
                        hf[r * FLAT_G:r * FLAT_G + R * HEXTW]
                        .rearrange("(t p c) -> p t c", p=128, c=HEXTW))
                for h in range(HEADS):
                    nc.sync.dma_start(
                        elT[h][0:2, :].rearrange("k (r n) -> k r n", r=NC_),
                        hfr[:, OFF_ELT + 2 * h * R:OFF_ELT + (2 * h + 2) * R]
                        .rearrange("r (k n) -> k r n", k=2))
            with tc.tile_pool(name=f"g{l}cg", bufs=1, space="PSUM") as psG:
              with tc.tile_pool(name=f"g{l}c", bufs=3, space="PSUM") as psC, \
                 tc.tile_pool(name=f"g{l}cs", bufs=3) as sbC:
                aggs = [psG.tile([65, R], F32, tag=f"agg{h}", name=f"agg{h}")
                        for h in range(HEADS)]
                for jt in range(JT):
                    jsl = slice(jt * 128, (jt + 1) * 128)
                    for h in range(HEADS):
                        pS = psC.tile([128, R], F32, tag="pS")
                        nc.tensor.matmul(pS[:], elT[h][:, jsl], erT[h][:],
                                         start=True, stop=True)
                        t1 = sbC.tile([128, R], BF16, tag="t1")
                        nc.vector.tensor_scalar_mul(t1[:], pS[:], 0.2)
                        t2 = sbC.tile([128, R], F32, tag="t2")
                        nc.vector.scalar_tensor_tensor(t2[:], pS[:], 1.0, t1[:],
                                                       op0=ALU.mult, op1=ALU.max)
                        w0 = sbC.tile([128, R], BF16, tag="w0")
                        nc.scalar.activation(w0[:], t2[:], AF.Exp, bias=0.0, scale=1.0)
                        w = sbC.tile([128, R], BF16, tag="w")
                        nc.gpsimd.tensor_tensor(w[:], w0[:], C_s[:, jt, :], ALU.mult)
                        nc.tensor.matmul(aggs[h][:], Haug_s[:, jt, 65 * h:65 * h + 65],
                                         w[:], start=(jt == 0), stop=(jt == JT - 1))
              with tc.tile_pool(name=f"g{l}d", bufs=2, space="PSUM") as psD, \
                 tc.tile_pool(name=f"g{l}ds", bufs=2) as sbD:
                for h in range(HEADS):
                    agg_s = sbD.tile([65, R], F32, tag="agg_s")
                    nc.vector.tensor_copy(agg_s[:], aggs[h][:])
                    zeps = sbD.tile([1, R], F32, tag="zeps")
                    nc.vector.tensor_scalar_add(zeps[:], agg_s[64:65, :], EPS)
                    pZ = psD.tile([64, R], F32, tag="pZ")
                    nc.tensor.matmul(pZ[:], ones1f[:], zeps[:], start=True, stop=True)
                    zin = sbD.tile([64, R], F32, tag="zin")
                    nc.vector.reciprocal(zin[:], pZ[:])
                    p0 = 64 * (h & 1)
                    kc = h >> 1
                    nc.vector.tensor_tensor(xgT[p0:p0 + 64, kc, :], agg_s[0:64, :],
                                            zin[:], ALU.mult)
                    nc.vector.tensor_copy(xgTbf[p0:p0 + 64, kc, :],
                                          xgT[p0:p0 + 64, kc, :])
            th = THETA[l]
            with tc.tile_pool(name=f"g{l}e", bufs=2, space="PSUM") as psE, \
                 tc.tile_pool(name=f"g{l}es", bufs=2) as sbF:
                for m in range(KC):
                    pg = psE.tile([128, R], F32, tag="pg")
                    for kc in range(4):
                        rhs_ = xgTbf[:, kc, :] if kc < KC else h0Tbf[:, kc - KC, :]
                        nc.tensor.matmul(pg[:], gcW_s[:, l, kc, m * 128:(m + 1) * 128],
                                         rhs_, start=(kc == 0), stop=(kc == 3))
                    u = sbF.tile([128, R], F32, tag="u")
                    nc.vector.scalar_tensor_tensor(u[:], pg[:], th, xT[:, m, :],
                                                   op0=ALU.mult, op1=ALU.add)
                    nc.vector.scalar_tensor_tensor(
                        u[:], xgT[:, m, :], (1.0 - th) * (1.0 - ALPHA), u[:],
                        op0=ALU.mult, op1=ALU.add)
                    nc.vector.scalar_tensor_tensor(
                        xT[:, m, :], h0T[:, m, :], (1.0 - th) * ALPHA, u[:],
                        op0=ALU.mult, op1=ALU.add)
                    nc.vector.tensor_copy(xTbf[:, m, :], xT[:, m, :])

        gatstk.close()

        # ---------------- P2: y_hat -> expB ----------------
        with tc.tile_pool(name="p2ps", bufs=2, space="PSUM") as p2ps, \
             tc.tile_pool(name="p2tr", bufs=2, space="PSUM") as p2tr, \
             tc.tile_pool(name="p2w", bufs=1) as p2w, \
             tc.tile_pool(name="p2s", bufs=3) as p2s:
            Y4T = p2w.tile([4, N], BF16, tag="Y4T", name="Y4T")
            rhs4 = p2w.tile([4, R], BF16, tag="rhs4", name="rhs4")
            nc.vector.memset(Y4T[:], 1.0)
            nc.vector.memset(rhs4[0:2, :], 1.0)
            yown = p2w.tile([128, NT], F32, tag="yown")
            for t in range(NT):
                py = p2ps.tile([128, 1], F32, tag="py")
                for k in range(KC):
                    nc.tensor.matmul(py[:], xT[:, k, t * 128:(t + 1) * 128],
                                     wd_s[:, k:k + 1], start=(k == 0), stop=(k == KC - 1))
                nc.scalar.activation(yown[:, t:t + 1], py[:], AF.Sigmoid,
                                     bias=bdbc_s[:], scale=1.0)
            yhl = p2w.tile([128, NT, 2], BF16, tag="yhl")
            nc.vector.tensor_copy(yhl[:, :, 0:1], yown[:].unsqueeze(-1))
            nc.vector.scalar_tensor_tensor(yhl[:, :, 1:2],
                                           yhl[:, :, 0:1], -1.0,
                                           yown[:].unsqueeze(-1),
                                           op0=ALU.mult, op1=ALU.add)
            ynhl = p2w.tile([128, NT, 2], BF16, tag="ynhl")
            nc.vector.tensor_scalar_mul(ynhl[:], yhl[:], -1.0)
            y2loc = p2w.tile([2, R], BF16, tag="y2loc")
            yn2loc = p2w.tile([2, R], BF16, tag="yn2loc")
            for t in range(NT):
                tsl = slice(t * 128, (t + 1) * 128)
                pt1 = p2tr.tile([2, 128], BF16, tag="pt1")
                nc.tensor.transpose(pt1[:], yhl[:, t, :], Ibf[:])
                nc.vector.tensor_copy(y2loc[:, tsl], pt1[:])
                pt2 = p2tr.tile([2, 128], BF16, tag="pt2")
                nc.tensor.transpose(pt2[:], ynhl[:, t, :], Ibf[:])
                nc.vector.tensor_copy(yn2loc[:, tsl], pt2[:])
            nc.sync.dma_start(T["yt2_in"][:].rearrange("(k n) -> k n", k=2), y2loc[:])
            nc.sync.dma_start(rhs4[2:4, :], yn2loc[:])
            nc.gpsimd.collective_compute("AllGather", ALU.bypass, replica_groups=RG,
                                         ins=[T["yt2_in"].opt()],
                                         outs=[T["yt2_out"].opt()])
            nc.sync.dma_start(
                Y4T[0:2, :].rearrange("k (r n) -> k r n", r=NC_),
                T["yt2_out"][:].rearrange("(r k n) -> k r n", r=NC_, k=2))
            for jt in range(JT):
                jsl = slice(jt * 128, (jt + 1) * 128)
                pB = p2ps.tile([128, R], F32, tag="pB")
                nc.tensor.matmul(pB[:], Y4T[:, jsl], rhs4[:], start=True, stop=True)
                a1 = p2s.tile([128, R], BF16, tag="a1")
                nc.vector.tensor_scalar_mul(a1[:], pB[:], -1.0)
                ab = p2s.tile([128, R], F32, tag="ab")
                nc.vector.scalar_tensor_tensor(ab[:], pB[:], 1.0, a1[:],
                                               op0=ALU.mult, op1=ALU.max)
                pmt = p2s.tile([128, R], U8, tag="pmt")
                nc.sync.dma_start(pmt[:], T["pairT_d"][jsl, :])
                pmneg = p2s.tile([128, R], F32, tag="pmneg")
                nc.vector.tensor_scalar(pmneg[:], pmt[:], 0.0, NEG,
                                        op0=ALU.is_equal, op1=ALU.mult)
                bt = p2s.tile([128, R], F32, tag="bt")
                nc.vector.scalar_tensor_tensor(bt[:], ab[:], -1.0, pmneg[:],
                                               op0=ALU.mult, op1=ALU.add)
                eB = p2s.tile([128, R], BF16, tag="eB")
                nc.scalar.activation(eB[:], bt[:], AF.Exp, bias=0.0, scale=1.0)
                nc.sync.dma_start(T["expB_d"][jsl, :], eB[:])

        # ---------------- P3: MS layers ----------------
        msstk = contextlib.ExitStack()
        msp = msstk.enter_context(tc.tile_pool(name="msp", bufs=1))
        qT = msp.tile([128, KC, R], BF16, tag="qT", name="qT")
        hmsT = msp.tile([128, KC, R], BF16, tag="hmsT", name="hmsT")
        kT_full = msp.tile([128, KC, N], BF16, tag="kT_full", name="kT_full")
        Vaug_s = msp.tile([128, JT, VW], BF16, tag="Vaug", name="Vaug_s")
        Xi13 = msp.tile([13, R], BF16, tag="Xi13", name="Xi13")
        Xj13_loc = msp.tile([13, R], BF16, tag="Xj13", name="Xj13_loc")
        XjT_s = msp.tile([13, N], BF16, tag="XjT", name="XjT_s")
        xyz_own = msp.tile([128, NT, 3], F32, tag="xyz_own", name="xyz_own")
        sq_own = msp.tile([128, NT], F32, tag="sq_own", name="sq_own")
        xyzhl = msp.tile([128, NT, 6], BF16, tag="xyzhl", name="xyzhl")
        m2hl = msp.tile([128, NT, 6], BF16, tag="m2hl", name="m2hl")
        sqhl = msp.tile([128, NT, 2], BF16, tag="sqhl", name="sqhl")
        for l in range(_NM):
            kvi, kvo = T["kv_in"][l], T["kv_out"][l]
            with tc.tile_pool(name=f"m{l}q", bufs=2, space="PSUM") as psQ, \
                 tc.tile_pool(name=f"m{l}qt", bufs=2, space="PSUM") as psQT, \
                 tc.tile_pool(name=f"m{l}qs", bufs=2) as sbQ:
                if l == 0:
                    nc.sync.dma_start(
                        xyz_own[:], T["xyz0_d"][:].rearrange("(t p) c -> p t c", p=128))
                    for t in range(NT):
                        sqv = sbQ.tile([128, 3], F32, tag="sqv")
                        nc.vector.tensor_tensor(sqv[:], xyz_own[:, t, :],
                                                xyz_own[:, t, :], ALU.mult)
                        nc.vector.tensor_reduce(sq_own[:, t:t + 1], sqv[:],
                                                axis=AX.X, op=ALU.add)
                # hi/lo splits
                nc.vector.tensor_copy(xyzhl[:, :, 0:3], xyz_own[:])
                nc.vector.scalar_tensor_tensor(xyzhl[:, :, 3:6], xyzhl[:, :, 0:3],
                                               -1.0, xyz_own[:],
                                               op0=ALU.mult, op1=ALU.add)
                nc.vector.tensor_scalar_mul(m2hl[:], xyzhl[:], -2.0)
                nc.vector.tensor_copy(sqhl[:, :, 0:1], sq_own[:].unsqueeze(-1))
                nc.vector.scalar_tensor_tensor(sqhl[:, :, 1:2], sqhl[:, :, 0:1],
                                               -1.0, sq_own[:].unsqueeze(-1),
                                               op0=ALU.mult, op1=ALU.add)
                # k-proj (into kv flat first so the collective can start asap)
                for m in range(KC):
                    pk = psQ.tile([128, R], F32, tag="pk")
                    for k in range(KC):
                        nc.tensor.matmul(pk[:], msW_s["k"][:, l, k, m * 128:(m + 1) * 128],
                                         xTbf[:, k, :], start=(k == 0), stop=(k == KC - 1))
                    kbf = sbQ.tile([128, R], BF16, tag="kbf")
                    nc.scalar.activation(kbf[:], pk[:], AF.Identity,
                                         bias=biases["kb"][:, l, m:m + 1], scale=1.0)
                    nc.sync.dma_start(
                        kvi[OFF_K + m * 128 * R:OFF_K + (m + 1) * 128 * R]
                        .rearrange("(p n) -> p n", p=128), kbf[:])
                # v-proj + xyz cols
                for t in range(NT):
                    pv = psQ.tile([128, HID], F32, tag="pv")
                    for k in range(KC):
                        nc.tensor.matmul(pv[:], xTbf[:, k, t * 128:(t + 1) * 128],
                                         msW_s["v"][:, l, k, :],
                                         start=(k == 0), stop=(k == KC - 1))
                    vst = sbQ.tile([128, VW], BF16, tag="vst")
                    nc.vector.tensor_tensor(vst[:, 0:HID], pv[:], vbbc_s[:, l, :],
                                            ALU.add)
                    nc.vector.tensor_copy(vst[:, HID:HID + 6], xyzhl[:, t, :])
                    nc.vector.memset(vst[:, HID + 6:HID + 7], 1.0)
                    nc.sync.dma_start(
                        kvi[OFF_V + t * 128 * VW:OFF_V + (t + 1) * 128 * VW]
                        .rearrange("(p c) -> p c", p=128), vst[:])
                # xyz-aug j-side (K=13 lhsT rows) and i-side rhs
                asmj = sbQ.tile([128, NT, 13], BF16, tag="asmj")
                asmi = sbQ.tile([128, NT, 13], BF16, tag="asmi")

                def col(dst, c, srcv):
                    nc.vector.tensor_copy(dst[:, :, c:c + 1], srcv)

                col(asmj, 0, sqhl[:, :, 0:1])
                col(asmj, 1, sqhl[:, :, 1:2])
                nc.vector.memset(asmj[:, :, 2:4], 1.0)
                for c_, s_ in ((4, 0), (5, 3), (6, 0), (7, 1), (8, 4), (9, 1),
                               (10, 2), (11, 5), (12, 2)):
                    col(asmj, c_, xyzhl[:, :, s_:s_ + 1])
                nc.vector.memset(asmi[:, :, 0:2], 1.0)
                col(asmi, 2, sqhl[:, :, 0:1])
                col(asmi, 3, sqhl[:, :, 1:2])
                for c_, s_ in ((4, 0), (5, 0), (6, 3), (7, 1), (8, 1), (9, 4),
                               (10, 2), (11, 2), (12, 5)):
                    col(asmi, c_, m2hl[:, :, s_:s_ + 1])
                for t in range(NT):
                    tsl = slice(t * 128, (t + 1) * 128)
                    ptj = psQT.tile([13, 128], BF16, tag="ptj")
                    nc.tensor.transpose(ptj[:], asmj[:, t, :], Ibf[:])
                    nc.vector.tensor_copy(Xj13_loc[:, tsl], ptj[:])
                    pti = psQT.tile([13, 128], BF16, tag="pti")
                    nc.tensor.transpose(pti[:], asmi[:, t, :], Ibf[:])
                    nc.vector.tensor_copy(Xi13[:, tsl], pti[:])
                nc.sync.dma_start(
                    kvi[OFF_XJ:OFF_XJ + 13 * R].rearrange("(k n) -> k n", k=13),
                    Xj13_loc[:])
            nc.gpsimd.collective_compute("AllGather", ALU.bypass, replica_groups=RG,
                                         ins=[kvi.opt()], outs=[kvo.opt()])
            # q-proj overlaps the collective
            with tc.tile_pool(name=f"m{l}p", bufs=2, space="PSUM") as psP:
                for m in range(KC):
                    pq = psP.tile([128, R], F32, tag="pq")
                    for k in range(KC):
                        nc.tensor.matmul(pq[:], msW_s["q"][:, l, k, m * 128:(m + 1) * 128],
                                         xTbf[:, k, :], start=(k == 0), stop=(k == KC - 1))
                    nc.scalar.activation(qT[:, m, :], pq[:], AF.Identity,
                                         bias=biases["qb"][:, l, m:m + 1],
                                         scale=1.0 / 16.0)
            with tc.tile_pool(name=f"m{l}u", bufs=1) as sbU2:  # noqa: F841
                for r in range(NC_):
                    rb = r * FLAT_M
                    nc.sync.dma_start(
                        kT_full[:, :, r * R:(r + 1) * R],
                        kvo[rb + OFF_K:rb + OFF_K + KC * 128 * R]
                        .rearrange("(m p n) -> p m n", p=128, n=R))
                    nc.sync.dma_start(
                        Vaug_s[:, NT * r:NT * (r + 1), :],
                        kvo[rb + OFF_V:rb + OFF_V + NT * 128 * VW]
                        .rearrange("(t p c) -> p t c", p=128, c=VW))
                    nc.sync.dma_start(
                        XjT_s[:, r * R:(r + 1) * R],
                        kvo[rb + OFF_XJ:rb + OFF_XJ + 13 * R]
                        .rearrange("(k n) -> k n", k=13))
            with tc.tile_pool(name=f"m{l}ro", bufs=1, space="PSUM") as psRO:
              pOs = [psRO.tile([128, VW], F32, tag=f"o{it}", name=f"pO{it}")
                     for it in range(NT)]
              with tc.tile_pool(name=f"m{l}r", bufs=2, space="PSUM") as psR, \
                 tc.tile_pool(name=f"m{l}rd", bufs=2, space="PSUM") as psRD, \
                 tc.tile_pool(name=f"m{l}rs", bufs=3) as sbR:
                for jt in range(JT):
                    jsl = slice(jt * 128, (jt + 1) * 128)
                    eBt = sbR.tile([128, R], BF16, tag="eBt")
                    nc.sync.dma_start(eBt[:], T["expB_d"][jsl, :])
                    pS = psR.tile([128, R], F32, tag="pS")
                    for k in range(KC):
                        nc.tensor.matmul(pS[:], kT_full[:, k, jsl], qT[:, k, :],
                                         start=(k == 0), stop=(k == KC - 1))
                    pD = psRD.tile([128, R], F32, tag="pD")
                    nc.tensor.matmul(pD[:], XjT_s[:, jsl], Xi13[:],
                                     start=True, stop=True)
                    rlu = sbR.tile([128, R], F32, tag="rlu")
                    nc.scalar.activation(rlu[:], pD[:], AF.Relu,
                                         bias=nbias[:], scale=BIGC)
                    sc = sbR.tile([128, R], BF16, tag="sc")
                    nc.vector.scalar_tensor_tensor(sc[:], rlu[:], -1.0, pS[:],
                                                   op0=ALU.mult, op1=ALU.add)
                    w0 = sbR.tile([128, R], BF16, tag="w0")
                    nc.scalar.activation(w0[:], sc[:], AF.Exp, bias=0.0, scale=1.0)
                    w = sbR.tile([128, R], BF16, tag="w")
                    nc.gpsimd.tensor_tensor(w[:], w0[:], eBt[:], ALU.mult)
                    for it in range(NT):
                        nc.tensor.matmul(pOs[it][:], w[:, it * 128:(it + 1) * 128],
                                         Vaug_s[:, jt, :],
                                         start=(jt == 0), stop=(jt == JT - 1))
              with tc.tile_pool(name=f"m{l}w", bufs=2, space="PSUM") as psW, \
                 tc.tile_pool(name=f"m{l}ws", bufs=2) as sbW:
                for it in range(NT):
                    isl = slice(it * 128, (it + 1) * 128)
                    pO = pOs[it]
                    zeps2 = sbW.tile([128, 1], F32, tag="zeps2")
                    nc.vector.tensor_scalar_add(zeps2[:], pO[:, VW - 1:VW], EPS)
                    zin2 = sbW.tile([128, 1], F32, tag="zin2")
                    nc.vector.reciprocal(zin2[:], zeps2[:])
                    hms = sbW.tile([128, HID], BF16, tag="hms")
                    nc.vector.tensor_scalar_mul(hms[:], pO[:, 0:HID], zin2[:])
                    for k in range(KC):
                        tph = psW.tile([128, 128], BF16, tag="tph")
                        nc.tensor.transpose(tph[:], hms[:, k * 128:(k + 1) * 128], Ibf[:])
                        nc.vector.tensor_copy(hmsT[:, k, isl], tph[:])
                    x6 = sbW.tile([128, 6], F32, tag="x6")
                    nc.vector.tensor_copy(x6[:], pO[:, HID:HID + 6])
                    xs = sbW.tile([128, 3], F32, tag="xs")
                    nc.vector.tensor_tensor(xs[:], x6[:, 0:3], x6[:, 3:6], ALU.add)
                    nc.vector.tensor_scalar_mul(xyz_own[:, it, :], xs[:], zin2[:])
                    sqv2 = sbW.tile([128, 3], F32, tag="sqv2")
                    nc.vector.tensor_tensor(sqv2[:], xyz_own[:, it, :],
                                            xyz_own[:, it, :], ALU.mult)
                    nc.vector.tensor_reduce(sq_own[:, it:it + 1], sqv2[:],
                                            axis=AX.X, op=ALU.add)
            with tc.tile_pool(name=f"m{l}o", bufs=2, space="PSUM") as psO2:
                for m in range(KC):
                    pp = psO2.tile([128, R], F32, tag="pp")
                    for k in range(KC):
                        nc.tensor.matmul(pp[:], msW_s["o"][:, l, k, m * 128:(m + 1) * 128],
                                         hmsT[:, k, :], start=(k == 0), stop=(k == KC - 1))
                    nc.vector.scalar_tensor_tensor(
                        xT[:, m, :], pp[:], biases["ob"][:, l, m:m + 1], xT[:, m, :],
                        op0=ALU.add, op1=ALU.add)
                    nc.vector.tensor_copy(xTbf[:, m, :], xT[:, m, :])

        msstk.close()

        # ---------------- P4: final logits ----------------
        with tc.tile_pool(name="p4ps", bufs=2, space="PSUM") as p4ps, \
             tc.tile_pool(name="p4s", bufs=2) as p4s:
            for t in range(NT):
                pf = p4ps.tile([128, 2], F32, tag="pf")
                for k in range(KC):
                    nc.tensor.matmul(pf[:], xT[:, k, t * 128:(t + 1) * 128],
                                     clsW_s[:, k, :], start=(k == 0), stop=(k == KC - 1))
                ot = p4s.tile([128, 2], F32, tag="ot")
                nc.vector.tensor_tensor(ot[:], pf[:], clsbbc_s[:], ALU.add)
                nc.sync.dma_start(T["out_d"][t * 128:(t + 1) * 128, :], ot[:])


# ================= entry point =================

def kernel(**inputs) -> np.ndarray:
    in_maps = _prep_host(inputs)
    if 0 not in _CACHE:
        _CACHE[0] = _build_program()
    nc = _CACHE[0]
    res = run_bass_kernel_spmd(nc, in_maps, list(range(NC_)))
    out = np.concatenate([res.results[c]["out"] for c in range(NC_)], axis=0)
    return np.ascontiguousarray(out.astype(np.float32))


# revision 9
# speedup vs baseline: 2.0847x; 1.0419x over previous
"""Trainium2 Bass kernel for nn_GAT_MS (GAT+GCNII stack -> mean-shift attention stack).

Dense formulation: GAT edge-softmax is computed as a dense masked [N x R]
transposed score matrix per head (rank-4 bf16 matmuls from hi/lo-split
attention logits), with edge multiplicity folded in as a count-matrix
multiply after exp. Aggregation is a dense matmul with a ones-column to
carry the softmax denominator. The mean-shift layers use the same
transposed-score layout (scores[j,i]) so no attention transpose is needed;
the distance-band test rides a K=13 bf16 hi/lo rank matmul and the
delta-y/pair-mask bias is pre-exponentiated (expB) so it folds in as a
single gpsimd multiply.

Self-contained: takes full inputs, shards nodes across 8 NeuronCores,
runs one SPMD Bass/Tile program via run_bass_kernel_spmd, gathers output.
"""

import sys

try:
    import concourse.bass as _b  # noqa: F401
except ImportError:
    sys.path.insert(0, "/opt/trn_rl_repo")

import contextlib
import os as _os
import numpy as np
import ml_dtypes

import concourse.bass as bass  # noqa: F401
import concourse.bacc as bacc
import concourse.tile as tile
import concourse.mybir as mybir
from concourse.bass_utils import run_bass_kernel_spmd

F32 = mybir.dt.float32
BF16 = mybir.dt.bfloat16
U8 = mybir.dt.uint8
AF = mybir.ActivationFunctionType
ALU = mybir.AluOpType
AX = mybir.AxisListType

# ---- problem constants (hardcoded) ----
N = 4096
FEAT = 64
HID = 256
HEADS = 4
DH = 64
L_GNN = 4
L_MS = 4
LAMDA = 0.5
ALPHA = 0.1
NEG = -1e9
EPS = 1e-9
BIGC = 3.0e7          # distance-band relu scale
BAND2 = 100.0         # band_width^2

NC_ = 8               # cores
R = N // NC_          # rows per core = 512
NT = R // 128         # node tiles per core = 4
JT = N // 128         # j tiles = 32
KC = HID // 128       # hid chunks = 2

# GAT hext layout: per node row [H0|1|H1|1|H2|1|H3|1|el8] = 268 cols
HEXTW = HEADS * (DH + 1) + 8          # 268
OFF_ELT = R * HEXTW                   # elT8 [8, R] row-major
FLAT_G = R * HEXTW + 8 * R

# MS kv flat layout (bf16)
VW = HID + 7                          # v(256)|xyz_hi(3)|xyz_lo(3)|1 = 263
OFF_K = 0
OFF_V = KC * 128 * R                  # 131072
OFF_XJ = OFF_V + NT * 128 * VW        # 265728
FLAT_M = OFF_XJ + 13 * R              # 272384

_NG = int(_os.environ.get("GATMS_NG", L_GNN))
_NM = int(_os.environ.get("GATMS_NM", L_MS))

_CACHE = {}


# ================= host-side preprocessing =================

def _bf(x):
    return np.ascontiguousarray(np.asarray(x, np.float32).astype(ml_dtypes.bfloat16))


def _f32(x):
    return np.ascontiguousarray(np.asarray(x, np.float32))


def _prep_host(inputs):
    feat = _f32(inputs["feat"])
    xyz = _f32(inputs["xyz"])
    pair = (np.asarray(inputs["distance_mask"]) &
            np.asarray(inputs["big_inter_mask"])).astype(np.uint8)
    src = np.asarray(inputs["src"]).astype(np.int64)
    dst = np.asarray(inputs["dst"]).astype(np.int64)
    cnt = np.zeros((N, N), np.int32)
    np.add.at(cnt, (dst, src), 1)
    cntT = _bf(cnt.T)                                    # [src j, dst i]

    stat = {}
    stat["fcW"] = _f32(inputs["fc_W"])
    stat["fcb"] = _f32(inputs["fc_b"]).reshape(HID)
    stat["gatW"] = _bf(inputs["gat_W"])
    al = _f32(inputs["attn_l"]).reshape(L_GNN, 1, HID)
    ar = _f32(inputs["attn_r"]).reshape(L_GNN, 1, HID)
    stat["albc"] = _f32(np.broadcast_to(al, (L_GNN, 128, HID)))
    stat["arbc"] = _f32(np.broadcast_to(ar, (L_GNN, 128, HID)))
    stat["gcW"] = _bf(inputs["gcnii_W"])
    cgW = _f32(inputs["cls_gat_W"])
    cgb = _f32(inputs["cls_gat_b"])
    stat["wd"] = _f32(cgW[:, 1] - cgW[:, 0])
    stat["bdbc"] = _f32(np.full((128, 1), float(cgb[1] - cgb[0])))
    for nm in ("q", "k", "v", "o"):
        stat[nm + "W"] = _bf(inputs[f"ms_{nm}_W"])
    stat["qb"] = _f32(inputs["ms_q_b"]) / 16.0
    stat["kb"] = _f32(inputs["ms_k_b"])
    stat["ob"] = _f32(inputs["ms_o_b"])
    vb = _f32(inputs["ms_v_b"]).reshape(L_MS, 1, HID)
    stat["vbbc"] = _f32(np.broadcast_to(vb, (L_MS, 128, HID)))
    stat["clsW"] = _f32(inputs["cls_W"])
    clsb = _f32(inputs["cls_b"]).reshape(1, 2)
    stat["clsbbc"] = _f32(np.broadcast_to(clsb, (128, 2)))
    stat["Ibf"] = _bf(np.eye(128))
    stat["If32"] = _f32(np.eye(128))

    in_maps = []
    for c in range(NC_):
        rows = slice(c * R, (c + 1) * R)
        m = dict(stat)
        m["featT"] = _f32(feat[rows].T)                  # [64, 512]
        m["xyz0"] = _f32(xyz[rows])                      # [512, 3]
        m["pairT"] = np.ascontiguousarray(pair[rows].T)  # [4096, 512] u8
        m["cntT"] = np.ascontiguousarray(cntT[:, rows])  # [4096, 512] bf16
        in_maps.append(m)
    return in_maps


# ================= device program =================

def _build_program():
    nc = bacc.Bacc("TRN2", target_bir_lowering=False, debug=False, num_devices=NC_)

    def din(name, shape, dt):
        return nc.dram_tensor(name, list(shape), dt, kind="ExternalInput").ap()

    T = {}
    T["featT_d"] = din("featT", (FEAT, R), F32)
    T["xyz0_d"] = din("xyz0", (R, 3), F32)
    T["pairT_d"] = din("pairT", (N, R), U8)
    T["cntT_d"] = din("cntT", (N, R), BF16)
    T["fcW_d"] = din("fcW", (FEAT, HID), F32)
    T["fcb_d"] = din("fcb", (HID,), F32)
    T["gatW_d"] = din("gatW", (L_GNN, HID, HID), BF16)
    T["albc_d"] = din("albc", (L_GNN, 128, HID), F32)
    T["arbc_d"] = din("arbc", (L_GNN, 128, HID), F32)
    T["gcW_d"] = din("gcW", (L_GNN, 2 * HID, HID), BF16)
    T["wd_d"] = din("wd", (HID,), F32)
    T["bdbc_d"] = din("bdbc", (128, 1), F32)
    for nm in ("q", "k", "v", "o"):
        T[nm + "W_d"] = din(nm + "W", (L_MS, HID, HID), BF16)
    T["qb_d"] = din("qb", (L_MS, HID), F32)
    T["kb_d"] = din("kb", (L_MS, HID), F32)
    T["ob_d"] = din("ob", (L_MS, HID), F32)
    T["vbbc_d"] = din("vbbc", (L_MS, 128, HID), F32)
    T["clsW_d"] = din("clsW", (HID, 2), F32)
    T["clsbbc_d"] = din("clsbbc", (128, 2), F32)
    T["Ibf_d"] = din("Ibf", (128, 128), BF16)
    T["If32_d"] = din("If32", (128, 128), F32)

    T["out_d"] = nc.dram_tensor("out", [R, 2], F32, kind="ExternalOutput").ap()

    T["hext"] = [nc.dram_tensor(f"hext{l}", [FLAT_G], BF16).ap()
                 for l in range(L_GNN)]
    T["hfull"] = [nc.dram_tensor(f"hfull{l}", [NC_ * FLAT_G], BF16,
                                 addr_space="Shared").ap() for l in range(L_GNN)]
    T["yt2_in"] = nc.dram_tensor("yt2_in", [2 * R], BF16).ap()
    T["yt2_out"] = nc.dram_tensor("yt2_out", [NC_ * 2 * R], BF16,
                                  addr_space="Shared").ap()
    T["expB_d"] = nc.dram_tensor("expB", [N, R], BF16).ap()
    T["kv_in"] = [nc.dram_tensor(f"kv_in{l}", [FLAT_M], BF16).ap()
                  for l in range(L_MS)]
    T["kv_out"] = [nc.dram_tensor(f"kv_out{l}", [NC_ * FLAT_M], BF16,
                                  addr_space="Shared").ap() for l in range(L_MS)]

    THETA = [min(1.0, float(np.log(LAMDA / (l + 1) + 1.0))) for l in range(L_GNN)]

    with tile.TileContext(nc) as tc:
        _emit(nc, tc, THETA, T)
    nc.compile()
    return nc


def _emit(nc, tc, THETA, T):
    RG = [list(range(NC_))]
    with contextlib.ExitStack() as ctx:
        pers = ctx.enter_context(tc.tile_pool(name="pers", bufs=1))

        def ptile(shape, dt, tag, src=None):
            t_ = pers.tile(list(shape), dt, tag=tag)
            if src is not None:
                nc.sync.dma_start(t_[:], src)
            return t_

        Ibf = ptile([128, 128], BF16, "Ibf", T["Ibf_d"][:])
        If32 = ptile([128, 128], F32, "If32", T["If32_d"][:])
        gatW_s = ptile([128, L_GNN, KC, HID], BF16, "gatW",
                       T["gatW_d"][:].rearrange("l (k p) h -> p l k h", p=128))
        albc_s = ptile([128, L_GNN, HID], F32, "albc",
                       T["albc_d"][:].rearrange("l p h -> p l h"))
        arbc_s = ptile([128, L_GNN, HID], F32, "arbc",
                       T["arbc_d"][:].rearrange("l p h -> p l h"))
        gcW_s = ptile([128, L_GNN, 4, HID], BF16, "gcW",
                      T["gcW_d"][:].rearrange("l (k p) h -> p l k h", p=128))
        msW_s = {}
        for nm in ("q", "k", "v", "o"):
            w_ = pers.tile([128, L_MS, KC, HID], BF16, tag=nm + "W", name=nm + "W_s")
            nc.sync.dma_start(w_[:], T[nm + "W_d"][:]
                              .rearrange("l (k p) h -> p l k h", p=128))
            msW_s[nm] = w_
        biases = {}
        for nm in ("qb", "kb", "ob"):
            b_ = pers.tile([128, L_MS, KC], F32, tag=nm, name=nm + "_s")
            nc.sync.dma_start(b_[:], T[nm + "_d"][:]
                              .rearrange("l (k p) -> p l k", p=128))
            biases[nm] = b_
        vbbc_s = ptile([128, L_MS, HID], F32, "vbbc",
                       T["vbbc_d"][:].rearrange("l p h -> p l h"))
        fcb_s = ptile([128, KC], F32, "fcb",
                      T["fcb_d"][:].rearrange("(k p) -> p k", p=128))
        clsW_s = ptile([128, KC, 2], F32, "clsW",
                       T["clsW_d"][:].rearrange("(k p) c -> p k c", p=128))
        clsbbc_s = ptile([128, 2], F32, "clsbbc", T["clsbbc_d"][:])
        wd_s = ptile([128, KC], F32, "wd",
                     T["wd_d"][:].rearrange("(k p) -> p k", p=128))
        bdbc_s = ptile([128, 1], F32, "bdbc", T["bdbc_d"][:])
        # state (persistent)
        xT = ptile([128, KC, R], F32, "xT")
        xTbf = ptile([128, KC, R], BF16, "xTbf")
        h0T = ptile([128, KC, R], F32, "h0T")
        h0Tbf = ptile([128, KC, R], BF16, "h0Tbf")
        xgT = ptile([128, KC, R], F32, "xgT")
        xgTbf = ptile([128, KC, R], BF16, "xgTbf")
        ones1f = ptile([1, 64], F32, "ones1f")
        nc.vector.memset(ones1f[:], 1.0)
        nbias = ptile([128, 1], F32, "nbias")
        nc.vector.memset(nbias[:], -BAND2 * BIGC)

        p0stk = contextlib.ExitStack()
        p0pool = p0stk.enter_context(tc.tile_pool(name="p0pool", bufs=1))
        featT_s = p0pool.tile([FEAT, R], F32, tag="featT", name="featT_s")
        nc.sync.dma_start(featT_s[:], T["featT_d"][:])
        fcW_s = p0pool.tile([FEAT, HID], F32, tag="fcW", name="fcW_s")
        nc.sync.dma_start(fcW_s[:], T["fcW_d"][:])

        # ---------------- P0: fc + relu ----------------
        with tc.tile_pool(name="p0ps", bufs=2, space="PSUM") as p0ps:
            for k in range(KC):
                ps = p0ps.tile([128, R], F32, tag="p0")
                nc.tensor.matmul(ps[:], fcW_s[:, k * 128:(k + 1) * 128], featT_s[:],
                                 start=True, stop=True)
                nc.scalar.activation(h0T[:, k, :], ps[:], AF.Relu,
                                     bias=fcb_s[:, k:k + 1], scale=1.0)
                nc.vector.tensor_copy(xT[:, k, :], h0T[:, k, :])
                nc.vector.tensor_copy(xTbf[:, k, :], h0T[:, k, :])
                nc.vector.tensor_copy(h0Tbf[:, k, :], h0T[:, k, :])

        p0stk.close()

        # ---------------- P1: GAT + GCNII ----------------
        gatstk = contextlib.ExitStack()
        gatp = gatstk.enter_context(tc.tile_pool(name="gatp", bufs=1))
        C_s = gatp.tile([128, JT, R], BF16, tag="C_s", name="C_s")
        nc.sync.dma_start(C_s[:], T["cntT_d"][:].rearrange("(t p) n -> p t n", p=128))
        Haug_s = gatp.tile([128, JT, HEXTW], BF16, tag="Haug", name="Haug_s")
        elT = [gatp.tile([4, N], BF16, tag=f"elT{h}", name=f"elT{h}")
               for h in range(HEADS)]
        erT = [gatp.tile([4, R], BF16, tag=f"erT{h}", name=f"erT{h}")
               for h in range(HEADS)]
        for h in range(HEADS):
            nc.vector.memset(elT[h][:], 1.0)
            nc.vector.memset(erT[h][0:2, :], 1.0)
        for l in range(_NG):
            hx, hf = T["hext"][l], T["hfull"][l]
            with tc.tile_pool(name=f"g{l}a", bufs=2, space="PSUM") as psA, \
                 tc.tile_pool(name=f"g{l}at", bufs=2, space="PSUM") as psAT, \
                 tc.tile_pool(name=f"g{l}as", bufs=2) as sbA, \
                 tc.tile_pool(name=f"g{l}ae", bufs=1) as sbE:
                el8 = sbE.tile([128, NT, 8], BF16, tag="el8")
                er8 = sbE.tile([128, NT, 8], BF16, tag="er8")
                elT8 = sbE.tile([8, R], BF16, tag="elT8")
                erT8 = sbE.tile([8, R], BF16, tag="erT8")
                for t in range(NT):
                    tsl = slice(t * 128, (t + 1) * 128)
                    ph = psA.tile([128, HID], F32, tag="ph")
                    for k in range(KC):
                        nc.tensor.matmul(ph[:], xTbf[:, k, tsl],
                                         gatW_s[:, l, k, :],
                                         start=(k == 0), stop=(k == KC - 1))
                    tmp = sbA.tile([128, HID], F32, tag="tmp")
                    e4 = sbA.tile([128, 2, HEADS], F32, tag="e4")
                    nc.vector.tensor_tensor(tmp[:], ph[:], albc_s[:, l, :], ALU.mult)
                    nc.vector.tensor_reduce(
                        e4[:, 0, :], tmp[:].rearrange("p (h d) -> p h d", h=HEADS),
                        axis=AX.X, op=ALU.add)
                    nc.vector.tensor_tensor(tmp[:], ph[:], arbc_s[:, l, :], ALU.mult)
                    nc.vector.tensor_reduce(
                        e4[:, 1, :], tmp[:].rearrange("p (h d) -> p h d", h=HEADS),
                        axis=AX.X, op=ALU.add)
                    ehi = sbA.tile([128, 2, HEADS], BF16, tag="ehi")
                    nc.vector.tensor_copy(ehi[:], e4[:])
                    elo = sbA.tile([128, 2, HEADS], BF16, tag="elo")
                    nc.vector.scalar_tensor_tensor(elo[:], ehi[:], -1.0, e4[:],
                                                   op0=ALU.mult, op1=ALU.add)
                    e8v = el8[:, t, :].rearrange("p (h k) -> p h k", k=2)
                    nc.vector.tensor_copy(e8v[:, :, 0:1], ehi[:, 0, :].unsqueeze(-1))
                    nc.vector.tensor_copy(e8v[:, :, 1:2], elo[:, 0, :].unsqueeze(-1))
                    r8v = er8[:, t, :].rearrange("p (h k) -> p h k", k=2)
                    nc.vector.tensor_copy(r8v[:, :, 0:1], ehi[:, 1, :].unsqueeze(-1))
                    nc.vector.tensor_copy(r8v[:, :, 1:2], elo[:, 1, :].unsqueeze(-1))
                    stage = sbA.tile([128, HEXTW], BF16, tag="stage")
                    for h in range(HEADS):
                        nc.vector.tensor_copy(
                            stage[:, h * 65:h * 65 + 64], ph[:, h * 64:(h + 1) * 64])
                        nc.vector.memset(stage[:, h * 65 + 64:h * 65 + 65], 1.0)
                    nc.vector.tensor_copy(stage[:, 260:268], el8[:, t, :])
                    nc.sync.dma_start(
                        hx[t * 128 * HEXTW:(t + 1) * 128 * HEXTW]
                        .rearrange("(p c) -> p c", p=128), stage[:])
                for t in range(NT):
                    tsl = slice(t * 128, (t + 1) * 128)
                    pt1 = psAT.tile([8, 128], BF16, tag="pt1")
                    nc.tensor.transpose(pt1[:], el8[:, t, :], Ibf[:])
                    nc.vector.tensor_copy(elT8[:, tsl], pt1[:])
                    pt2 = psAT.tile([8, 128], BF16, tag="pt2")
                    nc.tensor.transpose(pt2[:], er8[:, t, :], Ibf[:])
                    nc.vector.tensor_copy(erT8[:, tsl], pt2[:])
                nc.sync.dma_start(
                    hx[OFF_ELT:OFF_ELT + 8 * R].rearrange("(k n) -> k n", k=8),
                    elT8[:])
                for h in range(HEADS):
                    nc.sync.dma_start(erT[h][2:4, :], erT8[2 * h:2 * h + 2, :])
            nc.gpsimd.collective_compute("AllGather", ALU.bypass, replica_groups=RG,
                                         ins=[hx.opt()], outs=[hf.opt()])
            with tc.tile_pool(name=f"g{l}u", bufs=1) as sbU:  # noqa: F841
                hfr = hf.rearrange("(r x) -> r x", r=NC_)
                for r in range(NC_):
                    nc.sync.dma_start(
                        Haug_s[:, NT * r:NT * (r + 1), :],
                        hf[r * FLAT_G:r * FLAT_G + R * HEXTW]
                        .rearrange("(t p c) -> p t c", p=128, c=HEXTW))
                for h in range(HEADS):
                    nc.sync.dma_start(
                        elT[h][0:2, :].rearrange("k (r n) -> k r n", r=NC_),
                        hfr[:, OFF_ELT + 2 * h * R:OFF_ELT + (2 * h + 2) * R]
                        .rearrange("r (k n) -> k r n", k=2))
            with tc.tile_pool(name=f"g{l}cg", bufs=1, space="PSUM") as psG:
              with tc.tile_pool(name=f"g{l}c", bufs=3, space="PSUM") as psC, \
                 tc.tile_pool(name=f"g{l}cs", bufs=3) as sbC:
                aggs = [psG.tile([65, R], F32, tag=f"agg{h}", name=f"agg{h}")
                        for h in range(HEADS)]
                for jt in range(JT):
                    jsl = slice(jt * 128, (jt + 1) * 128)
                    for h in range(HEADS):
                        pS = psC.tile([128, R], F32, tag="pS")
                        nc.tensor.matmul(pS[:], elT[h][:, jsl], erT[h][:],
                                         start=True, stop=True)
                        t2 = sbC.tile([128, R], F32, tag="t2")
                        if h < 2:
                            t1 = sbC.tile([128, R], BF16, tag="t1")
                            nc.vector.tensor_scalar_mul(t1[:], pS[:], 0.2)
                            nc.vector.scalar_tensor_tensor(t2[:], pS[:], 1.0, t1[:],
                                                           op0=ALU.mult, op1=ALU.max)
                        else:
                            nc.scalar.activation(t2[:], pS[:], AF.Prelu,
                                                 bias=0.0, scale=1.0, alpha=0.2)
                        w0 = sbC.tile([128, R], BF16, tag="w0")
                        nc.scalar.activation(w0[:], t2[:], AF.Exp, bias=0.0, scale=1.0)
                        w = sbC.tile([128, R], BF16, tag="w")
                        if h == 0:
                            nc.vector.tensor_tensor(w[:], w0[:], C_s[:, jt, :], ALU.mult)
                        else:
                            nc.gpsimd.tensor_tensor(w[:], w0[:], C_s[:, jt, :], ALU.mult)
                        nc.tensor.matmul(aggs[h][:], Haug_s[:, jt, 65 * h:65 * h + 65],
                                         w[:], start=(jt == 0), stop=(jt == JT - 1))
              with tc.tile_pool(name=f"g{l}d", bufs=2, space="PSUM") as psD, \
                 tc.tile_pool(name=f"g{l}ds", bufs=2) as sbD:
                for h in range(HEADS):
                    agg_s = sbD.tile([65, R], F32, tag="agg_s")
                    nc.vector.tensor_copy(agg_s[:], aggs[h][:])
                    zeps = sbD.tile([1, R], F32, tag="zeps")
                    nc.vector.tensor_scalar_add(zeps[:], agg_s[64:65, :], EPS)
                    pZ = psD.tile([64, R], F32, tag="pZ")
                    nc.tensor.matmul(pZ[:], ones1f[:], zeps[:], start=True, stop=True)
                    zin = sbD.tile([64, R], F32, tag="zin")
                    nc.vector.reciprocal(zin[:], pZ[:])
                    p0 = 64 * (h & 1)
                    kc = h >> 1
                    nc.vector.tensor_tensor(xgT[p0:p0 + 64, kc, :], agg_s[0:64, :],
                                            zin[:], ALU.mult)
                    nc.vector.tensor_copy(xgTbf[p0:p0 + 64, kc, :],
                                          xgT[p0:p0 + 64, kc, :])
            th = THETA[l]
            with tc.tile_pool(name=f"g{l}e", bufs=2, space="PSUM") as psE, \
                 tc.tile_pool(name=f"g{l}es", bufs=2) as sbF:
                for m in range(KC):
                    pg = psE.tile([128, R], F32, tag="pg")
                    for kc in range(4):
                        rhs_ = xgTbf[:, kc, :] if kc < KC else h0Tbf[:, kc - KC, :]
                        nc.tensor.matmul(pg[:], gcW_s[:, l, kc, m * 128:(m + 1) * 128],
                                         rhs_, start=(kc == 0), stop=(kc == 3))
                    u = sbF.tile([128, R], F32, tag="u")
                    nc.vector.scalar_tensor_tensor(u[:], pg[:], th, xT[:, m, :],
                                                   op0=ALU.mult, op1=ALU.add)
                    nc.vector.scalar_tensor_tensor(
                        u[:], xgT[:, m, :], (1.0 - th) * (1.0 - ALPHA), u[:],
                        op0=ALU.mult, op1=ALU.add)
                    nc.vector.scalar_tensor_tensor(
                        xT[:, m, :], h0T[:, m, :], (1.0 - th) * ALPHA, u[:],
                        op0=ALU.mult, op1=ALU.add)
                    nc.vector.tensor_copy(xTbf[:, m, :], xT[:, m, :])

        gatstk.close()

        # ---------------- P2: y_hat -> expB ----------------
        with tc.tile_pool(name="p2ps", bufs=2, space="PSUM") as p2ps, \
             tc.tile_pool(name="p2tr", bufs=2, space="PSUM") as p2tr, \
             tc.tile_pool(name="p2w", bufs=1) as p2w, \
             tc.tile_pool(name="p2s", bufs=3) as p2s:
            Y4T = p2w.tile([4, N], BF16, tag="Y4T", name="Y4T")
            rhs4 = p2w.tile([4, R], BF16, tag="rhs4", name="rhs4")
            nc.vector.memset(Y4T[:], 1.0)
            nc.vector.memset(rhs4[0:2, :], 1.0)
            yown = p2w.tile([128, NT], F32, tag="yown")
            for t in range(NT):
                py = p2ps.tile([128, 1], F32, tag="py")
                for k in range(KC):
                    nc.tensor.matmul(py[:], xT[:, k, t * 128:(t + 1) * 128],
                                     wd_s[:, k:k + 1], start=(k == 0), stop=(k == KC - 1))
                nc.scalar.activation(yown[:, t:t + 1], py[:], AF.Sigmoid,
                                     bias=bdbc_s[:], scale=1.0)
            yhl = p2w.tile([128, NT, 2], BF16, tag="yhl")
            nc.vector.tensor_copy(yhl[:, :, 0:1], yown[:].unsqueeze(-1))
            nc.vector.scalar_tensor_tensor(yhl[:, :, 1:2],
                                           yhl[:, :, 0:1], -1.0,
                                           yown[:].unsqueeze(-1),
                                           op0=ALU.mult, op1=ALU.add)
            ynhl = p2w.tile([128, NT, 2], BF16, tag="ynhl")
            nc.vector.tensor_scalar_mul(ynhl[:], yhl[:], -1.0)
            y2loc = p2w.tile([2, R], BF16, tag="y2loc")
            yn2loc = p2w.tile([2, R], BF16, tag="yn2loc")
            for t in range(NT):
                tsl = slice(t * 128, (t + 1) * 128)
                pt1 = p2tr.tile([2, 128], BF16, tag="pt1")
                nc.tensor.transpose(pt1[:], yhl[:, t, :], Ibf[:])
                nc.vector.tensor_copy(y2loc[:, tsl], pt1[:])
                pt2 = p2tr.tile([2, 128], BF16, tag="pt2")
                nc.tensor.transpose(pt2[:], ynhl[:, t, :], Ibf[:])
                nc.vector.tensor_copy(yn2loc[:, tsl], pt2[:])
            nc.sync.dma_start(T["yt2_in"][:].rearrange("(k n) -> k n", k=2), y2loc[:])
            nc.sync.dma_start(rhs4[2:4, :], yn2loc[:])
            nc.gpsimd.collective_compute("AllGather", ALU.bypass, replica_groups=RG,
                                         ins=[T["yt2_in"].opt()],
                                         outs=[T["yt2_out"].opt()])
            nc.sync.dma_start(
                Y4T[0:2, :].rearrange("k (r n) -> k r n", r=NC_),
                T["yt2_out"][:].rearrange("(r k n) -> k r n", r=NC_, k=2))
            for jt in range(JT):
                jsl = slice(jt * 128, (jt + 1) * 128)
                pB = p2ps.tile([128, R], F32, tag="pB")
                nc.tensor.matmul(pB[:], Y4T[:, jsl], rhs4[:], start=True, stop=True)
                ab = p2s.tile([128, R], F32, tag="ab")
                nc.scalar.activation(ab[:], pB[:], AF.Prelu,
                                     bias=0.0, scale=1.0, alpha=-1.0)
                pmt = p2s.tile([128, R], U8, tag="pmt")
                nc.sync.dma_start(pmt[:], T["pairT_d"][jsl, :])
                pmneg = p2s.tile([128, R], F32, tag="pmneg")
                nc.vector.tensor_scalar(pmneg[:], pmt[:], 0.0, NEG,
                                        op0=ALU.is_equal, op1=ALU.mult)
                bt = p2s.tile([128, R], F32, tag="bt")
                nc.vector.scalar_tensor_tensor(bt[:], ab[:], -1.0, pmneg[:],
                                               op0=ALU.mult, op1=ALU.add)
                eB = p2s.tile([128, R], BF16, tag="eB")
                nc.scalar.activation(eB[:], bt[:], AF.Exp, bias=0.0, scale=1.0)
                nc.sync.dma_start(T["expB_d"][jsl, :], eB[:])

        # ---------------- P3: MS layers ----------------
        msstk = contextlib.ExitStack()
        msp = msstk.enter_context(tc.tile_pool(name="msp", bufs=1))
        qT = msp.tile([128, KC, R], BF16, tag="qT", name="qT")
        hmsT = msp.tile([128, KC, R], BF16, tag="hmsT", name="hmsT")
        kT_full = msp.tile([128, KC, N], BF16, tag="kT_full", name="kT_full")
        Vaug_s = msp.tile([128, JT, VW], BF16, tag="Vaug", name="Vaug_s")
        Xi13 = msp.tile([13, R], BF16, tag="Xi13", name="Xi13")
        Xj13_loc = msp.tile([13, R], BF16, tag="Xj13", name="Xj13_loc")
        XjT_s = msp.tile([13, N], BF16, tag="XjT", name="XjT_s")
        xyz_own = msp.tile([128, NT, 3], F32, tag="xyz_own", name="xyz_own")
        sq_own = msp.tile([128, NT], F32, tag="sq_own", name="sq_own")
        xyzhl = msp.tile([128, NT, 6], BF16, tag="xyzhl", name="xyzhl")
        m2hl = msp.tile([128, NT, 6], BF16, tag="m2hl", name="m2hl")
        sqhl = msp.tile([128, NT, 2], BF16, tag="sqhl", name="sqhl")
        for l in range(_NM):
            kvi, kvo = T["kv_in"][l], T["kv_out"][l]
            with tc.tile_pool(name=f"m{l}q", bufs=2, space="PSUM") as psQ, \
                 tc.tile_pool(name=f"m{l}qt", bufs=2, space="PSUM") as psQT, \
                 tc.tile_pool(name=f"m{l}qs", bufs=2) as sbQ:
                if l == 0:
                    nc.sync.dma_start(
                        xyz_own[:], T["xyz0_d"][:].rearrange("(t p) c -> p t c", p=128))
                    for t in range(NT):
                        sqv = sbQ.tile([128, 3], F32, tag="sqv")
                        nc.vector.tensor_tensor(sqv[:], xyz_own[:, t, :],
                                                xyz_own[:, t, :], ALU.mult)
                        nc.vector.tensor_reduce(sq_own[:, t:t + 1], sqv[:],
                                                axis=AX.X, op=ALU.add)
                # hi/lo splits
                nc.vector.tensor_copy(xyzhl[:, :, 0:3], xyz_own[:])
                nc.vector.scalar_tensor_tensor(xyzhl[:, :, 3:6], xyzhl[:, :, 0:3],
                                               -1.0, xyz_own[:],
                                               op0=ALU.mult, op1=ALU.add)
                nc.vector.tensor_scalar_mul(m2hl[:], xyzhl[:], -2.0)
                nc.vector.tensor_copy(sqhl[:, :, 0:1], sq_own[:].unsqueeze(-1))
                nc.vector.scalar_tensor_tensor(sqhl[:, :, 1:2], sqhl[:, :, 0:1],
                                               -1.0, sq_own[:].unsqueeze(-1),
                                               op0=ALU.mult, op1=ALU.add)
                # k-proj (into kv flat first so the collective can start asap)
                for m in range(KC):
                    pk = psQ.tile([128, R], F32, tag="pk")
                    for k in range(KC):
                        nc.tensor.matmul(pk[:], msW_s["k"][:, l, k, m * 128:(m + 1) * 128],
                                         xTbf[:, k, :], start=(k == 0), stop=(k == KC - 1))
                    kbf = sbQ.tile([128, R], BF16, tag="kbf")
                    nc.scalar.activation(kbf[:], pk[:], AF.Identity,
                                         bias=biases["kb"][:, l, m:m + 1], scale=1.0)
                    nc.sync.dma_start(
                        kvi[OFF_K + m * 128 * R:OFF_K + (m + 1) * 128 * R]
                        .rearrange("(p n) -> p n", p=128), kbf[:])
                # v-proj + xyz cols
                for t in range(NT):
                    pv = psQ.tile([128, HID], F32, tag="pv")
                    for k in range(KC):
                        nc.tensor.matmul(pv[:], xTbf[:, k, t * 128:(t + 1) * 128],
                                         msW_s["v"][:, l, k, :],
                                         start=(k == 0), stop=(k == KC - 1))
                    vst = sbQ.tile([128, VW], BF16, tag="vst")
                    nc.vector.tensor_tensor(vst[:, 0:HID], pv[:], vbbc_s[:, l, :],
                                            ALU.add)
                    nc.vector.tensor_copy(vst[:, HID:HID + 6], xyzhl[:, t, :])
                    nc.vector.memset(vst[:, HID + 6:HID + 7], 1.0)
                    nc.sync.dma_start(
                        kvi[OFF_V + t * 128 * VW:OFF_V + (t + 1) * 128 * VW]
                        .rearrange("(p c) -> p c", p=128), vst[:])
                # xyz-aug j-side (K=13 lhsT rows) and i-side rhs
                asmj = sbQ.tile([128, NT, 13], BF16, tag="asmj")
                asmi = sbQ.tile([128, NT, 13], BF16, tag="asmi")

                def col(dst, c, srcv):
                    nc.vector.tensor_copy(dst[:, :, c:c + 1], srcv)

                col(asmj, 0, sqhl[:, :, 0:1])
                col(asmj, 1, sqhl[:, :, 1:2])
                nc.vector.memset(asmj[:, :, 2:4], 1.0)
                for c_, s_ in ((4, 0), (5, 3), (6, 0), (7, 1), (8, 4), (9, 1),
                               (10, 2), (11, 5), (12, 2)):
                    col(asmj, c_, xyzhl[:, :, s_:s_ + 1])
                nc.vector.memset(asmi[:, :, 0:2], 1.0)
                col(asmi, 2, sqhl[:, :, 0:1])
                col(asmi, 3, sqhl[:, :, 1:2])
                for c_, s_ in ((4, 0), (5, 0), (6, 3), (7, 1), (8, 1), (9, 4),
                               (10, 2), (11, 2), (12, 5)):
                    col(asmi, c_, m2hl[:, :, s_:s_ + 1])
                for t in range(NT):
                    tsl = slice(t * 128, (t + 1) * 128)
                    ptj = psQT.tile([13, 128], BF16, tag="ptj")
                    nc.tensor.transpose(ptj[:], asmj[:, t, :], Ibf[:])
                    nc.vector.tensor_copy(Xj13_loc[:, tsl], ptj[:])
                    pti = psQT.tile([13, 128], BF16, tag="pti")
                    nc.tensor.transpose(pti[:], asmi[:, t, :], Ibf[:])
                    nc.vector.tensor_copy(Xi13[:, tsl], pti[:])
                nc.sync.dma_start(
                    kvi[OFF_XJ:OFF_XJ + 13 * R].rearrange("(k n) -> k n", k=13),
                    Xj13_loc[:])
            nc.gpsimd.collective_compute("AllGather", ALU.bypass, replica_groups=RG,
                                         ins=[kvi.opt()], outs=[kvo.opt()])
            # q-proj overlaps the collective
            with tc.tile_pool(name=f"m{l}p", bufs=2, space="PSUM") as psP:
                for m in range(KC):
                    pq = psP.tile([128, R], F32, tag="pq")
                    for k in range(KC):
                        nc.tensor.matmul(pq[:], msW_s["q"][:, l, k, m * 128:(m + 1) * 128],
                                         xTbf[:, k, :], start=(k == 0), stop=(k == KC - 1))
                    nc.scalar.activation(qT[:, m, :], pq[:], AF.Identity,
                                         bias=biases["qb"][:, l, m:m + 1],
                                         scale=1.0 / 16.0)
            with tc.tile_pool(name=f"m{l}u", bufs=1) as sbU2:  # noqa: F841
                for r in range(NC_):
                    rb = r * FLAT_M
                    nc.sync.dma_start(
                        kT_full[:, :, r * R:(r + 1) * R],
                        kvo[rb + OFF_K:rb + OFF_K + KC * 128 * R]
                        .rearrange("(m p n) -> p m n", p=128, n=R))
                    nc.sync.dma_start(
                        Vaug_s[:, NT * r:NT * (r + 1), :],
                        kvo[rb + OFF_V:rb + OFF_V + NT * 128 * VW]
                        .rearrange("(t p c) -> p t c", p=128, c=VW))
                    nc.sync.dma_start(
                        XjT_s[:, r * R:(r + 1) * R],
                        kvo[rb + OFF_XJ:rb + OFF_XJ + 13 * R]
                        .rearrange("(k n) -> k n", k=13))
            with tc.tile_pool(name=f"m{l}ro", bufs=1, space="PSUM") as psRO:
              pOs = [psRO.tile([128, VW], F32, tag=f"o{it}", name=f"pO{it}")
                     for it in range(NT)]
              with tc.tile_pool(name=f"m{l}r", bufs=2, space="PSUM") as psR, \
                 tc.tile_pool(name=f"m{l}rd", bufs=2, space="PSUM") as psRD, \
                 tc.tile_pool(name=f"m{l}rs", bufs=3) as sbR:
                for jt in range(JT):
                    jsl = slice(jt * 128, (jt + 1) * 128)
                    eBt = sbR.tile([128, R], BF16, tag="eBt")
                    nc.sync.dma_start(eBt[:], T["expB_d"][jsl, :])
                    pS = psR.tile([128, R], F32, tag="pS")
                    for k in range(KC):
                        nc.tensor.matmul(pS[:], kT_full[:, k, jsl], qT[:, k, :],
                                         start=(k == 0), stop=(k == KC - 1))
                    pD = psRD.tile([128, R], F32, tag="pD")
                    nc.tensor.matmul(pD[:], XjT_s[:, jsl], Xi13[:],
                                     start=True, stop=True)
                    rlu = sbR.tile([128, R], F32, tag="rlu")
                    nc.scalar.activation(rlu[:], pD[:], AF.Relu,
                                         bias=nbias[:], scale=BIGC)
                    sc = sbR.tile([128, R], BF16, tag="sc")
                    nc.vector.scalar_tensor_tensor(sc[:], rlu[:], -1.0, pS[:],
                                                   op0=ALU.mult, op1=ALU.add)
                    w0 = sbR.tile([128, R], BF16, tag="w0")
                    nc.scalar.activation(w0[:], sc[:], AF.Exp, bias=0.0, scale=1.0)
                    w = sbR.tile([128, R], BF16, tag="w")
                    nc.gpsimd.tensor_tensor(w[:], w0[:], eBt[:], ALU.mult)
                    for it in range(NT):
                        nc.tensor.matmul(pOs[it][:], w[:, it * 128:(it + 1) * 128],
                                         Vaug_s[:, jt, :],
                                         start=(jt == 0), stop=(jt == JT - 1))
              with tc.tile_pool(name=f"m{l}w", bufs=2, space="PSUM") as psW, \
                 tc.tile_pool(name=f"m{l}ws", bufs=2) as sbW:
                for it in range(NT):
                    isl = slice(it * 128, (it + 1) * 128)
                    pO = pOs[it]
                    zeps2 = sbW.tile([128, 1], F32, tag="zeps2")
                    nc.vector.tensor_scalar_add(zeps2[:], pO[:, VW - 1:VW], EPS)
                    zin2 = sbW.tile([128, 1], F32, tag="zin2")
                    nc.vector.reciprocal(zin2[:], zeps2[:])
                    hms = sbW.tile([128, HID], BF16, tag="hms")
                    nc.vector.tensor_scalar_mul(hms[:], pO[:, 0:HID], zin2[:])
                    for k in range(KC):
                        tph = psW.tile([128, 128], BF16, tag="tph")
                        nc.tensor.transpose(tph[:], hms[:, k * 128:(k + 1) * 128], Ibf[:])
                        nc.vector.tensor_copy(hmsT[:, k, isl], tph[:])
                    x6 = sbW.tile([128, 6], F32, tag="x6")
                    nc.vector.tensor_copy(x6[:], pO[:, HID:HID + 6])
                    xs = sbW.tile([128, 3], F32, tag="xs")
                    nc.vector.tensor_tensor(xs[:], x6[:, 0:3], x6[:, 3:6], ALU.add)
                    nc.vector.tensor_scalar_mul(xyz_own[:, it, :], xs[:], zin2[:])
                    sqv2 = sbW.tile([128, 3], F32, tag="sqv2")
                    nc.vector.tensor_tensor(sqv2[:], xyz_own[:, it, :],
                                            xyz_own[:, it, :], ALU.mult)
                    nc.vector.tensor_reduce(sq_own[:, it:it + 1], sqv2[:],
                                            axis=AX.X, op=ALU.add)
            with tc.tile_pool(name=f"m{l}o", bufs=2, space="PSUM") as psO2:
                for m in range(KC):
                    pp = psO2.tile([128, R], F32, tag="pp")
                    for k in range(KC):
                        nc.tensor.matmul(pp[:], msW_s["o"][:, l, k, m * 128:(m + 1) * 128],
                                         hmsT[:, k, :], start=(k == 0), stop=(k == KC - 1))
                    nc.vector.scalar_tensor_tensor(
                        xT[:, m, :], pp[:], biases["ob"][:, l, m:m + 1], xT[:, m, :],
                        op0=ALU.add, op1=ALU.add)
                    nc.vector.tensor_copy(xTbf[:, m, :], xT[:, m, :])

        msstk.close()

        # ---------------- P4: final logits ----------------
        with tc.tile_pool(name="p4ps", bufs=2, space="PSUM") as p4ps, \
             tc.tile_pool(name="p4s", bufs=2) as p4s:
            for t in range(NT):
                pf = p4ps.tile([128, 2], F32, tag="pf")
                for k in range(KC):
                    nc.tensor.matmul(pf[:], xT[:, k, t * 128:(t + 1) * 128],
                                     clsW_s[:, k, :], start=(k == 0), stop=(k == KC - 1))
                ot = p4s.tile([128, 2], F32, tag="ot")
                nc.vector.tensor_tensor(ot[:], pf[:], clsbbc_s[:], ALU.add)
                nc.sync.dma_start(T["out_d"][t * 128:(t + 1) * 128, :], ot[:])


# ================= entry point =================

def kernel(**inputs) -> np.ndarray:
    in_maps = _prep_host(inputs)
    if 0 not in _CACHE:
        _CACHE[0] = _build_program()
    nc = _CACHE[0]
    res = run_bass_kernel_spmd(nc, in_maps, list(range(NC_)))
    out = np.concatenate([res.results[c]["out"] for c in range(NC_)], axis=0)
    return np.ascontiguousarray(out.astype(np.float32))
